# revision 1
# baseline (speedup 1.0000x reference)
"""GPT-2-style causal attention block on 8 TRN2 NeuronCores (Bass/Tile).

Sharding (Megatron-style, per the hint): core c handles batch b = c // 4 and
head-group g = c % 4 (4 of the 16 heads).  Each core computes, fully locally:
  QKV projection (its 4 heads' columns), causal softmax attention for its
  4 heads, and the row-sharded output projection partial [S, D].
The host gathers by summing the 4 partials per batch and adding c_proj_b.

Per-core kernel layout choices:
  - x^T [D, S] is staged on host so Q^T/K^T come out of matmuls directly with
    head_dim on partitions (what the scores matmul wants) and V comes out in
    [seq, head_dim] (what the AV matmul wants).
  - scores are computed transposed, sT[j, i] (j = key index on partitions), so
    the exp'd tile is directly usable as the AV matmul's moving operand.
  - softmax uses exp without max subtraction (scores are O(1) here) and the
    denominator is computed by an extra ones-stationary matmul col-packed with
    the AV matmul, placed so numerator and denominator share partitions.
  - x/Wqkv/scores/probs/V run in bf16 (tile_position packing is illegal for
    4-byte dtypes); the output projection runs in float32r (TF32-rate).
  - attention is software-pipelined per (512-wide i-quarter, head pair):
    all score matmuls + one strided 2-head exp per key tile first, then the
    AV/rowsum accumulation chain, so PE never in-order-blocks behind ACT.
"""

from contextlib import ExitStack

import ml_dtypes
import numpy as np

B, S, D = 2, 2048, 1024
NH, HD = 16, 64
NCORES = 8
GROUPS = 4           # tensor-parallel head groups per batch
HPC = NH // GROUPS   # heads per core
SCALE = 1.0 / 8.0    # 1/sqrt(HD)

_CACHE = {}


def _body(ctx, tc, mybir, xt, wqk, wv, wp, qkb, vb, tri, onesb, out):
    nc = tc.nc
    f32 = mybir.dt.float32
    f32r = mybir.dt.float32r
    bf16 = mybir.dt.bfloat16
    EXP = mybir.ActivationFunctionType.Exp

    pin = ctx.enter_context(tc.tile_pool(name="pin", bufs=1))
    pwork = ctx.enter_context(tc.tile_pool(name="pwork", bufs=1))
    ppt = ctx.enter_context(tc.tile_pool(name="ppt", bufs=10))
    prec = ctx.enter_context(tc.tile_pool(name="prec", bufs=4))
    pstage = ctx.enter_context(tc.tile_pool(name="pstage", bufs=4))
    ps_mm = ctx.enter_context(tc.tile_pool(name="ps_mm", bufs=2, space="PSUM"))
    ps_s = ctx.enter_context(tc.tile_pool(name="ps_s", bufs=1, space="PSUM"))
    ps_av = ctx.enter_context(tc.tile_pool(name="ps_av", bufs=1, space="PSUM"))

    # ---------------- input staging ----------------
    xt_sb = pin.tile([128, 8 * 2048], bf16, name="xt_sb")
    wqk_sb = pin.tile([128, 4096], bf16, name="wqk_sb")
    for k in range(8):
        nc.sync.dma_start(wqk_sb[:, k * 512:(k + 1) * 512], wqk[:, k * 512:(k + 1) * 512])
        nc.sync.dma_start(xt_sb[:, k * 2048:(k + 1) * 2048], xt[k * 128:(k + 1) * 128, :])
    wv_sb = pin.tile([128, 2048], bf16, name="wv_sb")
    nc.sync.dma_start(wv_sb[:], wv[:])
    wp_sb = pin.tile([128, 2048], f32r, name="wp_sb")
    nc.sync.dma_start(wp_sb[:], wp[:])
    qkb_sb = pin.tile([128, 4], f32, name="qkb_sb")
    nc.sync.dma_start(qkb_sb[:], qkb[:])
    vb_sb = pin.tile([128, 256], f32, name="vb_sb")
    nc.sync.dma_start(vb_sb[:], vb[:])
    tri_sb = pin.tile([128, 128], bf16, name="tri_sb")
    nc.sync.dma_start(tri_sb[:], tri[:])

    # Q^T / K^T: head-pair p at cols [p*2048, (p+1)*2048); head hh of the pair
    # on partitions [hh*64, hh*64+64).
    qt_sb = pwork.tile([128, 2 * 2048], bf16, name="qt_sb")
    kt_sb = pwork.tile([128, 2 * 2048], bf16, name="kt_sb")
    # V: per j-tile block of 320 cols: [V_h0|V_h1|V_h2|V_h3|ones64]
    v_sb = pwork.tile([128, 16 * 320], bf16, name="v_sb")
    # a^T: k2 (head pair) at cols [k2*2048, ...), head hh on partitions hh*64..
    at_sb = pwork.tile([128, 2 * 2048], f32r, name="at_sb")

    nc.sync.dma_start(v_sb.rearrange("p (j c) -> p j c", c=320)[:, :, 256:320],
                      onesb.rearrange("p (j c) -> p j c", c=64))

    # Dummy exp so the ACT table set loads during the input-DMA window instead
    # of delaying the first real softmax exp.
    warm = pin.tile([128, 4], f32, name="warm")
    nc.scalar.activation(warm[:], qkb_sb[:], EXP, scale=0.0)

    # ---------------- QKV projection ----------------
    # col-tiles: C=0 -> Q pair0, C=1 -> Q pair1, C=2 -> K pair0, C=3 -> K pair1
    # k innermost across 4 live PSUM groups so compute starts as x^T tiles land.
    def qk_scblock(sc, Cs):
        tiles = {}
        for C in Cs:
            pool, tag = (ps_mm, "acc") if C < 2 else (ps_s, "s")
            tiles[C] = pool.tile([128, 512], f32, tag=tag, name=f"ps_qk{C}")
        for k in range(8):
            for C in Cs:
                nc.tensor.matmul(
                    tiles[C][:],
                    lhsT=wqk_sb[:, k * 512 + C * 128: k * 512 + C * 128 + 128],
                    rhs=xt_sb[:, k * 2048 + sc * 512: k * 2048 + sc * 512 + 512],
                    start=(k == 0), stop=(k == 7))
        for C in Cs:
            dest = qt_sb if C < 2 else kt_sb
            p = C % 2
            nc.vector.tensor_scalar_add(
                dest[:, p * 2048 + sc * 512: p * 2048 + (sc + 1) * 512],
                tiles[C][:], qkb_sb[:, C:C + 1])

    def v_jtile(j):
        ps = ps_mm.tile([128, 256], f32, tag="acc", name="ps_v")
        for k in range(8):
            nc.tensor.matmul(
                ps[:],
                lhsT=xt_sb[:, k * 2048 + j * 128: k * 2048 + (j + 1) * 128],
                rhs=wv_sb[:, k * 256:(k + 1) * 256],
                start=(k == 0), stop=(k == 7))
        nc.vector.tensor_add(v_sb[:, j * 320: j * 320 + 256], ps[:], vb_sb[:])

    for sc in range(4):
        qk_scblock(sc, (0, 2))
    for j in range(4):
        v_jtile(j)

    # ---------------- attention ----------------
    # Processed per (i-quarter Q of 512, head-pair p).  Scores for both heads
    # of the pair share one [128, 1024] PSUM tile (head hh at cols hh*512), so
    # a single strided exp covers both.  AV + softmax-denominator matmuls are
    # col-packed into two PSUM banks:
    #   bank A: rows 0:64 = a~_h0 (V-MM),    rows 64:128 = rowsum_h1 (ones-MM)
    #   bank B: rows 0:64 = rowsum_h0,       rows 64:128 = a~_h1
    # so each head's numerator and denominator land on the same partitions.
    def proj_stile(st):
        stage = pstage.tile([128, 1024], bf16, tag="stage", name="stage")
        for ec in range(2):
            ps = ps_mm.tile([128, 512], f32, tag="acc", name="ps_o")
            for k2 in range(2):
                nc.tensor.matmul(
                    ps[:],
                    lhsT=at_sb[:, k2 * 2048 + st * 128: k2 * 2048 + (st + 1) * 128],
                    rhs=wp_sb[:, k2 * 1024 + ec * 512: k2 * 1024 + (ec + 1) * 512],
                    start=(k2 == 0), stop=(k2 == 1))
            nc.vector.tensor_copy(stage[:, ec * 512:(ec + 1) * 512], ps[:])
        nc.sync.dma_start(out[st * 128:(st + 1) * 128, :], stage[:])

    DELAY = 3  # software-pipeline distance between scores/exp and AV use

    def att_qp(Q, p):
        qlo = Q * 512
        Jmax = 4 * Q + 3
        nJ = 4 * Q + 4
        ava = ps_av.tile([128, 512], f32, tag="ava", name="ava")
        avb = ps_av.tile([128, 512], f32, tag="avb", name="avb")
        h0 = p * 2
        pts = []
        for J in range(nJ + DELAY):
            if J < nJ:
                jlo = J * 128
                istart = max(jlo, qlo)
                w = qlo + 512 - istart
                pss = ps_s.tile([128, 1024], f32, tag="s", name="pss")
                for hh in range(2):
                    nc.tensor.matmul(
                        pss[:, hh * 512: hh * 512 + w],
                        lhsT=kt_sb[hh * 64:(hh + 1) * 64, p * 2048 + jlo: p * 2048 + jlo + 128],
                        rhs=qt_sb[hh * 64:(hh + 1) * 64, p * 2048 + istart: p * 2048 + istart + w],
                        start=True, stop=True)
                pt = ppt.tile([128, 1024], bf16, tag="pt", name="pt")
                nc.scalar.activation(
                    pt.rearrange("x (h c) -> x h c", c=512)[:, :, 0:w],
                    pss.rearrange("x (h c) -> x h c", c=512)[:, :, 0:w],
                    EXP, scale=SCALE)
                if jlo >= qlo:
                    # diagonal j-tile: zero the j > i triangle
                    nc.gpsimd.tensor_mul(pt[:, 0:128], pt[:, 0:128], tri_sb[:])
                    nc.gpsimd.tensor_mul(pt[:, 512:640], pt[:, 512:640], tri_sb[:])
                pts.append((pt, istart - qlo, w))
            Ja = J - DELAY
            if Ja < 0:
                continue
            pt, co, w = pts[Ja]
            ones = v_sb[:, Ja * 320 + 256: Ja * 320 + 320]
            v0 = v_sb[:, Ja * 320 + h0 * 64: Ja * 320 + h0 * 64 + 64]
            v1 = v_sb[:, Ja * 320 + (h0 + 1) * 64: Ja * 320 + (h0 + 1) * 64 + 64]
            r0 = pt[:, 0:w]
            r1 = pt[:, 512:512 + w]
            kw = dict(start=(Ja == 0), stop=(Ja == Jmax), skip_group_check=True)
            nc.tensor.matmul(ava[0:64, co:512], lhsT=v0, rhs=r0, **kw)
            nc.tensor.matmul(ava[64:128, co:512], lhsT=ones, rhs=r1, **kw)
            nc.tensor.matmul(avb[0:64, co:512], lhsT=ones, rhs=r0, **kw)
            nc.tensor.matmul(avb[64:128, co:512], lhsT=v1, rhs=r1, **kw)
        # normalize and write a^T
        rec = prec.tile([128, 512], f32, tag="rec", name="rec")
        nc.vector.reciprocal(rec[0:64, :], avb[0:64, :])
        nc.vector.reciprocal(rec[64:128, :], ava[64:128, :])
        nc.vector.tensor_mul(
            at_sb[0:64, p * 2048 + qlo: p * 2048 + qlo + 512],
            ava[0:64, :], rec[0:64, :])
        nc.vector.tensor_mul(
            at_sb[64:128, p * 2048 + qlo: p * 2048 + qlo + 512],
            avb[64:128, :], rec[64:128, :])

    # attention (Q0, pair0) is emitted early (in place of the old monolithic
    # QKV phase) so the ACT exp stream starts as soon as pair-0 Q/K exist.
    att_qp(0, 0)
    for sc in range(4):
        qk_scblock(sc, (1, 3))
    for j in range(4, 16):
        v_jtile(j)
    att_qp(0, 1)
    for Q in range(1, 4):
        att_qp(Q, 0)
        # previous quarter's projection: ready PE filler for this quarter's
        # ACT-paced attention stretch
        for st in range(4 * (Q - 1), 4 * Q):
            proj_stile(st)
        att_qp(Q, 1)
    for st in range(12, 16):
        proj_stile(st)

def _build_nc(repeat=1):
    key = ("nc", repeat)
    if key in _CACHE:
        return _CACHE[key]
    import concourse.bacc as bacc
    import concourse.mybir as mybir
    import concourse.tile as tile

    f32 = mybir.dt.float32
    f32r = mybir.dt.float32r
    bf16d = mybir.dt.bfloat16
    nc = bacc.Bacc("TRN2", target_bir_lowering=False, debug=False)
    xt = nc.dram_tensor("xt", [D, S], bf16d, kind="ExternalInput").ap()
    wqk = nc.dram_tensor("wqk", [128, 4096], bf16d, kind="ExternalInput").ap()
    wv = nc.dram_tensor("wv", [128, 2048], bf16d, kind="ExternalInput").ap()
    wp = nc.dram_tensor("wp", [128, 2048], f32r, kind="ExternalInput").ap()
    qkb = nc.dram_tensor("qkb", [128, 4], f32, kind="ExternalInput").ap()
    vb = nc.dram_tensor("vb", [128, 256], f32, kind="ExternalInput").ap()
    tri = nc.dram_tensor("tri", [128, 128], mybir.dt.bfloat16, kind="ExternalInput").ap()
    onesb = nc.dram_tensor("onesb", [128, 1024], mybir.dt.bfloat16, kind="ExternalInput").ap()
    out = nc.dram_tensor("out", [S, D], bf16d, kind="ExternalOutput").ap()

    with tile.TileContext(nc) as tc:
        for _ in range(repeat):
            with ExitStack() as ctx:
                _body(ctx, tc, mybir, xt, wqk, wv, wp, qkb, vb, tri, onesb, out)
    nc.compile()
    _CACHE[key] = nc
    return nc


def _make_in_maps(hidden_states, c_attn_w, c_attn_b, c_proj_w):
    hs = np.asarray(hidden_states, dtype=np.float32)
    waw = np.asarray(c_attn_w, dtype=np.float32)
    wab = np.asarray(c_attn_b, dtype=np.float32)
    wpw = np.asarray(c_proj_w, dtype=np.float32)

    tri = np.triu(np.ones((128, 128), dtype=ml_dtypes.bfloat16))
    xts = [np.ascontiguousarray(hs[b].T).astype(ml_dtypes.bfloat16) for b in range(B)]
    in_maps = []
    for c in range(NCORES):
        b, g = divmod(c, GROUPS)
        cols = np.arange(g * HPC * HD, (g + 1) * HPC * HD)
        wqk_host = np.concatenate([waw[:, cols], waw[:, D + cols]], axis=1)
        in_maps.append({
            "xt": xts[b],
            "wqk": np.ascontiguousarray(
                wqk_host.reshape(8, 128, 512).transpose(1, 0, 2).reshape(128, 4096)).astype(ml_dtypes.bfloat16),
            "wv": np.ascontiguousarray(
                waw[:, 2 * D + cols].reshape(8, 128, 256).transpose(1, 0, 2).reshape(128, 2048)).astype(ml_dtypes.bfloat16),
            "wp": np.ascontiguousarray(
                wpw[cols, :].reshape(2, 128, 1024).transpose(1, 0, 2).reshape(128, 2048)),
            "qkb": np.ascontiguousarray(
                np.concatenate([wab[cols], wab[D + cols]]).reshape(4, 128).T),
            "vb": np.ascontiguousarray(
                np.broadcast_to(wab[2 * D + cols], (128, 256))),
            "tri": tri,
            "onesb": np.ones((128, 1024), ml_dtypes.bfloat16),
        })
    return in_maps


def kernel(hidden_states, c_attn_w, c_attn_b, c_proj_w, c_proj_b):
    from concourse import bass_utils

    nc = _build_nc()
    in_maps = _make_in_maps(hidden_states, c_attn_w, c_attn_b, c_proj_w)
    res = bass_utils.run_bass_kernel_spmd(nc, in_maps, core_ids=list(range(NCORES)))
    outs = [np.asarray(r["out"], dtype=np.float32) for r in res.results]
    wpb = np.asarray(c_proj_b, dtype=np.float32)
    full = np.stack(
        [sum(outs[b * GROUPS:(b + 1) * GROUPS]) + wpb for b in range(B)], axis=0)
    return full.astype(np.float32)



# revision 21
# speedup vs baseline: 1.2848x; 1.2848x over previous
"""GPT-2-style causal attention block on 8 TRN2 NeuronCores (Bass/Tile).

Sharding (Megatron-style, per the hint): core c handles batch b = c // 4 and
head-group g = c % 4 (4 of the 16 heads).  Each core computes, fully locally:
  QKV projection (its 4 heads' columns), causal softmax attention for its
  4 heads, and the row-sharded output projection partial [S, D].
The host gathers by summing the 4 partials per batch and adding c_proj_b.

Per-core kernel layout choices:
  - x^T [D, S] is staged on host so Q^T/K^T come out of matmuls directly with
    head_dim on partitions (what the scores matmul wants) and V comes out in
    [seq, head_dim] (what the AV matmul wants).
  - scores are computed transposed, sT[j, i] (j = key index on partitions), so
    the exp'd tile is directly usable as the AV matmul's moving operand.
  - softmax denominator comes from the SAME matmul as AV: each head's V block
    carries a ones column ([V|1]), so psum row 64 is the rowsum and rows 0:64
    the numerator.  Normalization: DVE reciprocal of row 64, a 1-contraction
    matmul against a ones row (reused from tri) broadcasts it over 64
    partitions, then a DVE mul writes a^T.  The odd head's product lands in a
    temp tile at partitions 0:64 and is lane-shifted to at[64:128] by a tiny
    SBUF->SBUF DMA.  This halves attention PE work vs a separate ones-matmul.
  - QKV phase A runs k-outer while x^T/W stream from HBM: Q/K (i-halves
    sc0, sc1) and V (j0-3) accumulate in 8 live PSUM banks, so PE tracks the
    input DMA instead of idling.  x^T lands in 512-col quarters for finer
    pipelining.  The rest of QKV + the output projection are emitted as
    "filler" work units between attention J-steps, keeping PE busy while the
    exp stream paces softmax.
  - x/Wqkv/scores/probs/V run in bf16; the output projection in float32r.
"""

from contextlib import ExitStack

import ml_dtypes
import numpy as np

B, S, D = 2, 2048, 1024
NH, HD = 16, 64
NCORES = 8
GROUPS = 4           # tensor-parallel head groups per batch
HPC = NH // GROUPS   # heads per core
SCALE = 1.0 / 8.0    # 1/sqrt(HD)
VBLK = 260           # per-j-tile V block: [V0|1|V1|1|V2|1|V3|1]

_CACHE = {}


def _body(ctx, tc, mybir, xt, wqk, wv, wp, qkb, vb, tri, onesd, onesh, out):
    nc = tc.nc
    f32 = mybir.dt.float32
    f32r = mybir.dt.float32r
    bf16 = mybir.dt.bfloat16
    EXP = mybir.ActivationFunctionType.Exp

    pin = ctx.enter_context(tc.tile_pool(name="pin", bufs=1))
    pwork = ctx.enter_context(tc.tile_pool(name="pwork", bufs=1))
    ppt = ctx.enter_context(tc.tile_pool(name="ppt", bufs=10))
    prec = ctx.enter_context(tc.tile_pool(name="prec", bufs=4))
    pstage = ctx.enter_context(tc.tile_pool(name="pstage", bufs=4))
    ps_mm = ctx.enter_context(tc.tile_pool(name="ps_mm", bufs=2, space="PSUM"))
    ps_s = ctx.enter_context(tc.tile_pool(name="ps_s", bufs=2, space="PSUM"))
    ps_av = ctx.enter_context(tc.tile_pool(name="ps_av", bufs=1, space="PSUM"))

    # ---------------- input staging ----------------
    qkb_sb = pin.tile([128, 4], f32, name="qkb_sb")
    nc.sync.dma_start(qkb_sb[:], qkb[:])
    vb_sb = pin.tile([128, 256], f32, name="vb_sb")
    nc.sync.dma_start(vb_sb[:], vb[:])
    tri_sb = pin.tile([128, 128], bf16, name="tri_sb")
    nc.sync.dma_start(tri_sb[:], tri[:])

    xt_sb = pin.tile([128, 8 * 2048], bf16, name="xt_sb")
    wqk_sb = pin.tile([128, 4096], bf16, name="wqk_sb")
    wv_sb = pin.tile([128, 2048], bf16, name="wv_sb")
    # phase A consumes quarters q0/q1 of each chunk; q2/q3 stream afterwards.
    for k in range(8):
        nc.sync.dma_start(wqk_sb[:, k * 512:(k + 1) * 512], wqk[:, k * 512:(k + 1) * 512])
        nc.sync.dma_start(wv_sb[:, k * 256:(k + 1) * 256], wv[:, k * 256:(k + 1) * 256])
        for q in range(2):
            nc.sync.dma_start(
                xt_sb[:, k * 2048 + q * 512: k * 2048 + (q + 1) * 512],
                xt[k * 128:(k + 1) * 128, q * 512:(q + 1) * 512])
    for k in range(8):
        for q in range(2, 4):
            nc.sync.dma_start(
                xt_sb[:, k * 2048 + q * 512: k * 2048 + (q + 1) * 512],
                xt[k * 128:(k + 1) * 128, q * 512:(q + 1) * 512])
    wp_sb = pin.tile([128, 2048], f32r, name="wp_sb")
    nc.sync.dma_start(wp_sb[:], wp[:])

    # Q^T / K^T: head-pair p at cols [p*2048, (p+1)*2048); head hh of the pair
    # on partitions [hh*64, hh*64+64).
    qt_sb = pwork.tile([128, 2 * 2048], bf16, name="qt_sb")
    kt_sb = pwork.tile([128, 2 * 2048], bf16, name="kt_sb")
    # V blocks of VBLK cols per j-tile: head h's [V_h|1] at cols h*65
    v_sb = pwork.tile([128, 16 * VBLK], bf16, name="v_sb")
    # a^T: k2 (head pair) at cols [k2*2048, ...), head hh on partitions hh*64..
    at_sb = pwork.tile([128, 2 * 2048], f32r, name="at_sb")

    # f32r ones row for the reciprocal-broadcast matmuls + bf16 ones scattered
    # into the V blocks' 65th columns (memset fails the walrus ISA check, so
    # both come from a small DRAM constant).
    onesr = pin.tile([128, 64], f32r, name="onesr")
    nc.sync.dma_start(onesr[:], onesd[:])
    ones64 = pin.tile([128, 64], bf16, name="ones64")
    nc.sync.dma_start(ones64[:], onesh[:])
    nc.vector.tensor_copy(
        v_sb.rearrange("p (g c) -> p g c", c=65)[:, :, 64:65],
        ones64.rearrange("p (g c) -> p g c", c=1))

    # Dummy exp so the ACT table set loads during the input-DMA window instead
    # of delaying the first real softmax exp.
    warm = pin.tile([128, 4], f32, name="warm")
    nc.scalar.activation(warm[:], qkb_sb[:], EXP, scale=0.0)

    # ---------------- QKV helpers ----------------
    # col-tiles: C=0 -> Q pair0, C=1 -> Q pair1, C=2 -> K pair0, C=3 -> K pair1
    def qk_add(ps_ap, C, sc):
        dest = qt_sb if C < 2 else kt_sb
        p = C % 2
        nc.vector.tensor_scalar_add(
            dest[:, p * 2048 + sc * 512: p * 2048 + (sc + 1) * 512],
            ps_ap, qkb_sb[:, C:C + 1])

    def v_add(ps_ap256, j):
        dst = v_sb[:, j * VBLK:(j + 1) * VBLK].rearrange(
            "p (g c) -> p g c", c=65)[:, :, 0:64]
        nc.vector.tensor_add(
            dst,
            ps_ap256.rearrange("p (g c) -> p g c", c=64),
            vb_sb.rearrange("p (g c) -> p g c", c=64))

    # ---------------- phase A: k-outer QKV subset ----------------
    # 8 live PSUM banks track the input stream: Q/K for sc0 (acc0/acc1),
    # sc1 (ava/avb), V j0-3 packed into one 2-bank tile (s).
    a_q0 = ps_mm.tile([128, 512], f32, tag="acc", name="a_q0")
    a_k0 = ps_mm.tile([128, 512], f32, tag="acc", name="a_k0")
    a_q1 = ps_av.tile([128, 512], f32, tag="ava", name="a_q1")
    a_k1 = ps_av.tile([128, 512], f32, tag="avb", name="a_k1")
    a_v = ps_s.tile([128, 1024], f32, tag="s", name="a_v")
    for k in range(8):
        for (ps_t, C, sc) in ((a_q0, 0, 0), (a_k0, 2, 0)):
            nc.tensor.matmul(
                ps_t[:],
                lhsT=wqk_sb[:, k * 512 + C * 128: k * 512 + (C + 1) * 128],
                rhs=xt_sb[:, k * 2048 + sc * 512: k * 2048 + (sc + 1) * 512],
                start=(k == 0), stop=(k == 7))
        for j in range(4):
            # start=True zeroes the whole 2KB bank region, so only the first
            # group per bank (j=0 for cols 0:512, j=2 for 512:1024) may start.
            nc.tensor.matmul(
                a_v[:, j * 256:(j + 1) * 256],
                lhsT=xt_sb[:, k * 2048 + j * 128: k * 2048 + (j + 1) * 128],
                rhs=wv_sb[:, k * 256:(k + 1) * 256],
                start=(k == 0 and j % 2 == 0), stop=(k == 7),
                skip_group_check=True)
        for (ps_t, C, sc) in ((a_q1, 0, 1), (a_k1, 2, 1)):
            nc.tensor.matmul(
                ps_t[:],
                lhsT=wqk_sb[:, k * 512 + C * 128: k * 512 + (C + 1) * 128],
                rhs=xt_sb[:, k * 2048 + sc * 512: k * 2048 + (sc + 1) * 512],
                start=(k == 0), stop=(k == 7))
    qk_add(a_k0[:], 2, 0)
    qk_add(a_q0[:], 0, 0)
    for j in range(4):
        v_add(a_v[:, j * 256:(j + 1) * 256], j)
    qk_add(a_q1[:], 0, 1)
    qk_add(a_k1[:], 2, 1)

    # ---------------- filler work units ----------------
    def qk_block(sc, C):
        def go():
            ps = ps_mm.tile([128, 512], f32, tag="acc", name="qkB")
            for k in range(8):
                nc.tensor.matmul(
                    ps[:],
                    lhsT=wqk_sb[:, k * 512 + C * 128: k * 512 + (C + 1) * 128],
                    rhs=xt_sb[:, k * 2048 + sc * 512: k * 2048 + (sc + 1) * 512],
                    start=(k == 0), stop=(k == 7))
            qk_add(ps[:], C, sc)
        return go

    def v_block(j):
        def go():
            ps = ps_mm.tile([128, 256], f32, tag="acc", name="vB")
            for k in range(8):
                nc.tensor.matmul(
                    ps[:],
                    lhsT=xt_sb[:, k * 2048 + j * 128: k * 2048 + (j + 1) * 128],
                    rhs=wv_sb[:, k * 256:(k + 1) * 256],
                    start=(k == 0), stop=(k == 7))
            v_add(ps[:], j)
        return go

    def proj_stile(st):
        def go():
            stage = pstage.tile([128, 1024], bf16, tag="stage", name="stage")
            for ec in range(2):
                ps = ps_mm.tile([128, 512], f32, tag="acc", name="ps_o")
                for k2 in range(2):
                    nc.tensor.matmul(
                        ps[:],
                        lhsT=at_sb[:, k2 * 2048 + st * 128: k2 * 2048 + (st + 1) * 128],
                        rhs=wp_sb[:, k2 * 1024 + ec * 512: k2 * 1024 + (ec + 1) * 512],
                        start=(k2 == 0), stop=(k2 == 1))
                nc.vector.tensor_copy(stage[:, ec * 512:(ec + 1) * 512], ps[:])
            nc.sync.dma_start(out[st * 128:(st + 1) * 128, :], stage[:])
        return go

    filler = [qk_block(0, 1), qk_block(0, 3),
              qk_block(2, 0), qk_block(2, 2),
              qk_block(3, 0), qk_block(3, 2),
              qk_block(1, 1), qk_block(1, 3),
              v_block(4), v_block(5), v_block(6), v_block(7),
              qk_block(2, 1), qk_block(2, 3),
              v_block(8), v_block(9), v_block(10), v_block(11),
              qk_block(3, 1), qk_block(3, 3),
              v_block(12), v_block(13), v_block(14), v_block(15)]
    pend = []  # deferred attention finishers (normalize + a^T write)

    # ---------------- attention ----------------
    # Per (i-quarter Q of 512, head-pair p).  Scores for both heads of the
    # pair share one [128, 1024] PSUM tile (head hh at cols hh*512) so one
    # strided exp covers both.  AV psums (per head):
    #   av[0:65] = [V|1]^T probs  -> rows 0:64 numerator, row 64 rowsum
    # The finisher normalizes lane-aligned at partitions 0:64 and lane-shifts
    # the odd head's a^T to partitions 64:128 with a SBUF->SBUF DMA.
    DELAY = 3  # software-pipeline distance between scores/exp and AV use

    def att_qp(Q, p):
        qlo = Q * 512
        Jmax = 4 * Q + 3
        nJ = 4 * Q + 4
        ava = ps_av.tile([128, 512], f32, tag="ava", name="ava")
        avb = ps_av.tile([128, 512], f32, tag="avb", name="avb")
        pts = []
        for J in range(nJ + DELAY):
            if J < nJ:
                jlo = J * 128
                istart = max(jlo, qlo)
                w = qlo + 512 - istart
                pss = ps_s.tile([128, 1024], f32, tag="s", name="pss")
                for hh in range(2):
                    nc.tensor.matmul(
                        pss[:, hh * 512: hh * 512 + w],
                        lhsT=kt_sb[hh * 64:(hh + 1) * 64, p * 2048 + jlo: p * 2048 + jlo + 128],
                        rhs=qt_sb[hh * 64:(hh + 1) * 64, p * 2048 + istart: p * 2048 + istart + w],
                        start=True, stop=True)
                pt = ppt.tile([128, 1024], bf16, tag="pt", name="pt")
                nc.scalar.activation(
                    pt.rearrange("x (h c) -> x h c", c=512)[:, :, 0:w],
                    pss.rearrange("x (h c) -> x h c", c=512)[:, :, 0:w],
                    EXP, scale=SCALE)
                if jlo >= qlo:
                    # diagonal j-tile: zero the j > i triangle
                    nc.gpsimd.tensor_mul(pt[:, 0:128], pt[:, 0:128], tri_sb[:])
                    nc.gpsimd.tensor_mul(pt[:, 512:640], pt[:, 512:640], tri_sb[:])
                pts.append((pt, istart - qlo, w))
            if J == 0 and pend:
                pend.pop(0)()
            if J % 2 == 0 and filler:
                filler.pop(0)()
            Ja = J - DELAY
            if Ja < 0:
                continue
            pt, co, w = pts[Ja]
            base = Ja * VBLK + p * 130
            kw = dict(start=(Ja == 0), stop=(Ja == Jmax), skip_group_check=True)
            nc.tensor.matmul(ava[0:65, co:512], lhsT=v_sb[:, base: base + 65],
                             rhs=pt[:, 0:w], **kw)
            nc.tensor.matmul(avb[0:65, co:512], lhsT=v_sb[:, base + 65: base + 130],
                             rhs=pt[:, 512:512 + w], **kw)

        def finish():
            ones_row = onesr[64:65, :]
            rec = prec.tile([128, 1024], f32r, tag="rec", name="rec")
            with nc.allow_low_precision(reason="softmax denominators in f32r"):
                nc.vector.reciprocal(rec[64:65, 0:512], ava[64:65, :])
                nc.vector.reciprocal(rec[64:65, 512:1024], avb[64:65, :])
            # broadcast each head's reciprocal row over 64 partitions; DVE may
            # read only one PSUM operand, so stage the broadcast in SBUF.
            bc_sb = prec.tile([128, 1024], f32r, tag="bcs", name="bc_sb")
            for half in range(2):
                bc = ps_mm.tile([128, 512], f32, tag="acc", name="bc")
                nc.tensor.matmul(bc[0:64, :], lhsT=ones_row,
                                 rhs=rec[64:65, half * 512:(half + 1) * 512],
                                 start=True, stop=True)
                nc.vector.tensor_copy(bc_sb[0:64, half * 512:(half + 1) * 512],
                                      bc[0:64, :])
            nc.vector.tensor_mul(
                at_sb[0:64, p * 2048 + qlo: p * 2048 + qlo + 512],
                ava[0:64, :], bc_sb[0:64, 0:512])
            at_tmp = prec.tile([128, 512], f32r, tag="att", name="at_tmp")
            nc.vector.tensor_mul(at_tmp[0:64, :], avb[0:64, :],
                                 bc_sb[0:64, 512:1024])
            nc.sync.dma_start(
                at_sb[64:128, p * 2048 + qlo: p * 2048 + qlo + 512],
                at_tmp[0:64, :])
            if p == 1:
                filler.extend(proj_stile(st) for st in range(4 * Q, 4 * Q + 4))
        pend.append(finish)

    for Q in range(4):
        att_qp(Q, 0)
        att_qp(Q, 1)
    while pend:
        pend.pop(0)()
    while filler:
        filler.pop(0)()


def _build_nc(repeat=1):
    key = ("nc", repeat)
    if key in _CACHE:
        return _CACHE[key]
    import concourse.bacc as bacc
    import concourse.mybir as mybir
    import concourse.tile as tile

    f32 = mybir.dt.float32
    f32r = mybir.dt.float32r
    bf16d = mybir.dt.bfloat16
    nc = bacc.Bacc("TRN2", target_bir_lowering=False, debug=False)
    xt = nc.dram_tensor("xt", [D, S], bf16d, kind="ExternalInput").ap()
    wqk = nc.dram_tensor("wqk", [128, 4096], bf16d, kind="ExternalInput").ap()
    wv = nc.dram_tensor("wv", [128, 2048], bf16d, kind="ExternalInput").ap()
    wp = nc.dram_tensor("wp", [128, 2048], f32r, kind="ExternalInput").ap()
    qkb = nc.dram_tensor("qkb", [128, 4], f32, kind="ExternalInput").ap()
    vb = nc.dram_tensor("vb", [128, 256], f32, kind="ExternalInput").ap()
    tri = nc.dram_tensor("tri", [128, 128], bf16d, kind="ExternalInput").ap()
    onesd = nc.dram_tensor("onesd", [128, 64], f32r, kind="ExternalInput").ap()
    onesh = nc.dram_tensor("onesh", [128, 64], bf16d, kind="ExternalInput").ap()
    out = nc.dram_tensor("out", [S, D], bf16d, kind="ExternalOutput").ap()

    with tile.TileContext(nc) as tc:
        for _ in range(repeat):
            with ExitStack() as ctx:
                _body(ctx, tc, mybir, xt, wqk, wv, wp, qkb, vb, tri, onesd, onesh, out)
    nc.compile()
    _CACHE[key] = nc
    return nc


def _make_in_maps(hidden_states, c_attn_w, c_attn_b, c_proj_w):
    hs = np.asarray(hidden_states, dtype=np.float32)
    waw = np.asarray(c_attn_w, dtype=np.float32)
    wab = np.asarray(c_attn_b, dtype=np.float32)
    wpw = np.asarray(c_proj_w, dtype=np.float32)

    tri = np.triu(np.ones((128, 128), dtype=ml_dtypes.bfloat16))
    xts = [np.ascontiguousarray(hs[b].T).astype(ml_dtypes.bfloat16) for b in range(B)]
    in_maps = []
    for c in range(NCORES):
        b, g = divmod(c, GROUPS)
        cols = np.arange(g * HPC * HD, (g + 1) * HPC * HD)
        wqk_host = np.concatenate([waw[:, cols], waw[:, D + cols]], axis=1)
        in_maps.append({
            "xt": xts[b],
            "wqk": np.ascontiguousarray(
                wqk_host.reshape(8, 128, 512).transpose(1, 0, 2).reshape(128, 4096)).astype(ml_dtypes.bfloat16),
            "wv": np.ascontiguousarray(
                waw[:, 2 * D + cols].reshape(8, 128, 256).transpose(1, 0, 2).reshape(128, 2048)).astype(ml_dtypes.bfloat16),
            "wp": np.ascontiguousarray(
                wpw[cols, :].reshape(2, 128, 1024).transpose(1, 0, 2).reshape(128, 2048)),
            "qkb": np.ascontiguousarray(
                np.concatenate([wab[cols], wab[D + cols]]).reshape(4, 128).T),
            "vb": np.ascontiguousarray(
                np.broadcast_to(wab[2 * D + cols], (128, 256))),
            "tri": tri,
            "onesd": np.ones((128, 64), np.float32),
            "onesh": np.ones((128, 64), ml_dtypes.bfloat16),
        })
    return in_maps


def kernel(hidden_states, c_attn_w, c_attn_b, c_proj_w, c_proj_b):
    from concourse import bass_utils

    nc = _build_nc()
    in_maps = _make_in_maps(hidden_states, c_attn_w, c_attn_b, c_proj_w)
    res = bass_utils.run_bass_kernel_spmd(nc, in_maps, core_ids=list(range(NCORES)))
    outs = [np.asarray(r["out"], dtype=np.float32) for r in res.results]
    wpb = np.asarray(c_proj_b, dtype=np.float32)
    full = np.stack(
        [sum(outs[b * GROUPS:(b + 1) * GROUPS]) + wpb for b in range(B)], axis=0)
    return full.astype(np.float32)


# revision 28
# speedup vs baseline: 1.3692x; 1.0657x over previous
"""GPT-2-style causal attention block on 8 TRN2 NeuronCores (Bass/Tile).

Sharding (Megatron-style, per the hint): core c handles batch b = c // 4 and
head-group g = c % 4 (4 of the 16 heads).  Each core computes, fully locally:
  QKV projection (its 4 heads' columns), causal softmax attention for its
  4 heads, and the row-sharded output projection partial [S, D].
The host gathers by summing the 4 partials per batch and adding c_proj_b.

Per-core kernel layout choices:
  - x^T [D, S] is staged on host so Q^T/K^T come out of matmuls directly with
    head_dim on partitions (what the scores matmul wants) and V comes out in
    [seq, head_dim] (what the AV matmul wants).
  - scores are computed transposed, sT[j, i] (j = key index on partitions), so
    the exp'd tile is directly usable as the AV matmul's moving operand.
  - softmax denominator comes from the SAME matmul as AV: each head's V block
    carries a ones column ([V|1]), so psum row 64 is the rowsum and rows 0:64
    the numerator.  Normalization: DVE reciprocal of row 64, a 1-contraction
    matmul against a ones row (reused from tri) broadcasts it over 64
    partitions, then a DVE mul writes a^T.  The odd head's product lands in a
    temp tile at partitions 0:64 and is lane-shifted to at[64:128] by a tiny
    SBUF->SBUF DMA.  This halves attention PE work vs a separate ones-matmul.
  - QKV phase A runs k-outer while x^T/W stream from HBM: Q/K (i-halves
    sc0, sc1) and V (j0-3) accumulate in 8 live PSUM banks, so PE tracks the
    input DMA instead of idling.  x^T lands in 512-col quarters for finer
    pipelining.  The rest of QKV + the output projection are emitted as
    "filler" work units between attention J-steps, keeping PE busy while the
    exp stream paces softmax.
  - x/Wqkv/scores/probs/V run in bf16; the output projection in float32r.
"""

from contextlib import ExitStack

import ml_dtypes
import numpy as np

B, S, D = 2, 2048, 1024
NH, HD = 16, 64
NCORES = 8
GROUPS = 4           # tensor-parallel head groups per batch
HPC = NH // GROUPS   # heads per core
SCALE = 1.0 / 8.0    # 1/sqrt(HD)
VBLK = 260           # per-j-tile V block: [V0|1|V1|1|V2|1|V3|1]

_CACHE = {}


def _body(ctx, tc, mybir, xt, wqk, wv, wp, qkb, vb, tri, onesd, onesh, out):
    nc = tc.nc
    f32 = mybir.dt.float32
    f32r = mybir.dt.float32r
    bf16 = mybir.dt.bfloat16
    EXP = mybir.ActivationFunctionType.Exp

    pin = ctx.enter_context(tc.tile_pool(name="pin", bufs=1))
    pwork = ctx.enter_context(tc.tile_pool(name="pwork", bufs=1))
    ppt = ctx.enter_context(tc.tile_pool(name="ppt", bufs=10))
    prec = ctx.enter_context(tc.tile_pool(name="prec", bufs=4))
    pstage = ctx.enter_context(tc.tile_pool(name="pstage", bufs=4))
    ps_mm = ctx.enter_context(tc.tile_pool(name="ps_mm", bufs=2, space="PSUM"))
    ps_s = ctx.enter_context(tc.tile_pool(name="ps_s", bufs=2, space="PSUM"))
    ps_av = ctx.enter_context(tc.tile_pool(name="ps_av", bufs=1, space="PSUM"))

    # ---------------- input staging ----------------
    # Each DMA costs ~625ns of serial HWDGE time on top of its transfer, so
    # batch inputs into few, large DMAs ordered by first use.
    qkb_sb = pin.tile([128, 4], f32, name="qkb_sb")
    nc.sync.dma_start(qkb_sb[:], qkb[:])
    tri_sb = pin.tile([128, 128], bf16, name="tri_sb")
    nc.sync.dma_start(tri_sb[:], tri[:])

    xt_sb = pin.tile([128, 8 * 2048], bf16, name="xt_sb")
    wqk_sb = pin.tile([128, 4096], bf16, name="wqk_sb")
    wv_sb = pin.tile([128, 2048], bf16, name="wv_sb")
    nc.sync.dma_start(wqk_sb[:, 0:2048], wqk[:, 0:2048])
    nc.sync.dma_start(xt_sb[:, 0:2048], xt[0:128, :])
    nc.sync.dma_start(wv_sb[:], wv[:])
    for k in range(1, 4):
        nc.sync.dma_start(xt_sb[:, k * 2048:(k + 1) * 2048],
                          xt[k * 128:(k + 1) * 128, :])
    vb_sb = pin.tile([128, 256], f32, name="vb_sb")
    nc.sync.dma_start(vb_sb[:], vb[:])
    onesr = pin.tile([128, 64], f32r, name="onesr")
    nc.sync.dma_start(onesr[:], onesd[:])
    ones64 = pin.tile([128, 64], bf16, name="ones64")
    nc.sync.dma_start(ones64[:], onesh[:])
    nc.sync.dma_start(wqk_sb[:, 2048:4096], wqk[:, 2048:4096])
    for k in range(4, 8):
        nc.sync.dma_start(xt_sb[:, k * 2048:(k + 1) * 2048],
                          xt[k * 128:(k + 1) * 128, :])
    wp_sb = pin.tile([128, 2048], f32r, name="wp_sb")
    nc.sync.dma_start(wp_sb[:], wp[:])

    # Q^T / K^T: head-pair p at cols [p*2048, (p+1)*2048); head hh of the pair
    # on partitions [hh*64, hh*64+64).
    qt_sb = pwork.tile([128, 2 * 2048], bf16, name="qt_sb")
    kt_sb = pwork.tile([128, 2 * 2048], bf16, name="kt_sb")
    # V blocks of VBLK cols per j-tile: head h's [V_h|1] at cols h*65
    v_sb = pwork.tile([128, 16 * VBLK], bf16, name="v_sb")
    # a^T: k2 (head pair) at cols [k2*2048, ...), head hh on partitions hh*64..
    at_sb = pwork.tile([128, 2 * 2048], f32r, name="at_sb")

    # Dummy exp so the ACT table set loads during the input-DMA window instead
    # of delaying the first real softmax exp.
    warm = pin.tile([128, 4], f32, name="warm")
    nc.scalar.activation(warm[:], qkb_sb[:], EXP, scale=0.0)

    # ---------------- QKV helpers ----------------
    # col-tiles: C=0 -> Q pair0, C=1 -> Q pair1, C=2 -> K pair0, C=3 -> K pair1
    # psum->SBUF bias-add moves run on ACT (Copy is in the exp table set, and
    # ACT is idle during the QKV phases) to keep DVE free for softmax work.
    CPY = mybir.ActivationFunctionType.Copy
    IDN = mybir.ActivationFunctionType.Identity

    def qk_add(ps_ap, C, sc):
        dest = qt_sb if C < 2 else kt_sb
        p = C % 2
        nc.scalar.activation(
            dest[:, p * 2048 + sc * 512: p * 2048 + (sc + 1) * 512],
            ps_ap, IDN, bias=qkb_sb[:, C:C + 1])

    def v_add(ps_ap256, j):
        dst = v_sb[:, j * VBLK:(j + 1) * VBLK].rearrange(
            "p (g c) -> p g c", c=65)[:, :, 0:64]
        nc.vector.tensor_add(
            dst,
            ps_ap256.rearrange("p (g c) -> p g c", c=64),
            vb_sb.rearrange("p (g c) -> p g c", c=64))

    # ---------------- phase A: k-outer QKV subset ----------------
    # 8 live PSUM banks track the input stream: Q/K for sc0 (acc0/acc1),
    # sc1 (ava/avb), V j0-3 packed into one 2-bank tile (s).
    a_q0 = ps_mm.tile([128, 512], f32, tag="acc", name="a_q0")
    a_k0 = ps_mm.tile([128, 512], f32, tag="acc", name="a_k0")
    a_q1 = ps_av.tile([128, 512], f32, tag="ava", name="a_q1")
    a_k1 = ps_av.tile([128, 512], f32, tag="avb", name="a_k1")
    a_v = ps_s.tile([128, 1024], f32, tag="s", name="a_v")
    for k in range(8):
        for (ps_t, C, sc) in ((a_q0, 0, 0), (a_k0, 2, 0)):
            nc.tensor.matmul(
                ps_t[:],
                lhsT=wqk_sb[:, k * 512 + C * 128: k * 512 + (C + 1) * 128],
                rhs=xt_sb[:, k * 2048 + sc * 512: k * 2048 + (sc + 1) * 512],
                start=(k == 0), stop=(k == 7))
        for j in range(4):
            # start=True zeroes the whole 2KB bank region, so only the first
            # group per bank (j=0 for cols 0:512, j=2 for 512:1024) may start.
            nc.tensor.matmul(
                a_v[:, j * 256:(j + 1) * 256],
                lhsT=xt_sb[:, k * 2048 + j * 128: k * 2048 + (j + 1) * 128],
                rhs=wv_sb[:, k * 256:(k + 1) * 256],
                start=(k == 0 and j % 2 == 0), stop=(k == 7),
                skip_group_check=True)
        for (ps_t, C, sc) in ((a_q1, 0, 1), (a_k1, 2, 1)):
            nc.tensor.matmul(
                ps_t[:],
                lhsT=wqk_sb[:, k * 512 + C * 128: k * 512 + (C + 1) * 128],
                rhs=xt_sb[:, k * 2048 + sc * 512: k * 2048 + (sc + 1) * 512],
                start=(k == 0), stop=(k == 7))
    qk_add(a_k0[:], 2, 0)
    qk_add(a_q0[:], 0, 0)
    for j in range(4):
        v_add(a_v[:, j * 256:(j + 1) * 256], j)
    qk_add(a_q1[:], 0, 1)
    qk_add(a_k1[:], 2, 1)
    # ones columns of the V blocks (emitted after the v_adds so the DVE queue
    # is not head-blocked waiting for the onesh DMA; disjoint columns)
    nc.vector.tensor_copy(
        v_sb.rearrange("p (g c) -> p g c", c=65)[:, :, 64:65],
        ones64.rearrange("p (g c) -> p g c", c=1))

    # ---------------- filler work units ----------------
    def qk_block(sc, C):
        def go():
            ps = ps_mm.tile([128, 512], f32, tag="acc", name="qkB")
            for k in range(8):
                nc.tensor.matmul(
                    ps[:],
                    lhsT=wqk_sb[:, k * 512 + C * 128: k * 512 + (C + 1) * 128],
                    rhs=xt_sb[:, k * 2048 + sc * 512: k * 2048 + (sc + 1) * 512],
                    start=(k == 0), stop=(k == 7))
            qk_add(ps[:], C, sc)
        return go

    def v_block(j):
        def go():
            ps = ps_mm.tile([128, 256], f32, tag="acc", name="vB")
            for k in range(8):
                nc.tensor.matmul(
                    ps[:],
                    lhsT=xt_sb[:, k * 2048 + j * 128: k * 2048 + (j + 1) * 128],
                    rhs=wv_sb[:, k * 256:(k + 1) * 256],
                    start=(k == 0), stop=(k == 7))
            v_add(ps[:], j)
        return go

    def proj_stile(st, on_act=False):
        def go():
            stage = pstage.tile([128, 1024], bf16, tag="stage", name="stage")
            for ec in range(2):
                ps = ps_mm.tile([128, 512], f32, tag="acc", name="ps_o")
                for k2 in range(2):
                    nc.tensor.matmul(
                        ps[:],
                        lhsT=at_sb[:, k2 * 2048 + st * 128: k2 * 2048 + (st + 1) * 128],
                        rhs=wp_sb[:, k2 * 1024 + ec * 512: k2 * 1024 + (ec + 1) * 512],
                        start=(k2 == 0), stop=(k2 == 1))
                if on_act:  # tail stiles: ACT is idle once the exps are done
                    nc.scalar.activation(stage[:, ec * 512:(ec + 1) * 512],
                                         ps[:], CPY)
                else:
                    nc.vector.tensor_copy(stage[:, ec * 512:(ec + 1) * 512],
                                          ps[:])
            nc.sync.dma_start(out[st * 128:(st + 1) * 128, :], stage[:])
        return go

    filler = [qk_block(0, 1), qk_block(0, 3),
              qk_block(2, 0), qk_block(2, 2),
              qk_block(3, 0), qk_block(3, 2),
              qk_block(1, 1), qk_block(1, 3),
              v_block(4), v_block(5), v_block(6), v_block(7),
              qk_block(2, 1), qk_block(2, 3),
              v_block(8), v_block(9), v_block(10), v_block(11),
              qk_block(3, 1), qk_block(3, 3),
              v_block(12), v_block(13), v_block(14), v_block(15)]
    pend = []  # deferred attention finishers (normalize + a^T write)

    # ---------------- attention ----------------
    # Per (i-quarter Q of 512, head-pair p).  Scores for both heads of the
    # pair share one [128, 1024] PSUM tile (head hh at cols hh*512) so one
    # strided exp covers both.  AV psums (per head):
    #   av[0:65] = [V|1]^T probs  -> rows 0:64 numerator, row 64 rowsum
    # The finisher normalizes lane-aligned at partitions 0:64 and lane-shifts
    # the odd head's a^T to partitions 64:128 with a SBUF->SBUF DMA.
    DELAY = 3  # software-pipeline distance between scores/exp and AV use

    def att_qp(Q, p):
        qlo = Q * 512
        Jmax = 4 * Q + 3
        nJ = 4 * Q + 4
        ava = ps_av.tile([128, 512], f32, tag="ava", name="ava")
        avb = ps_av.tile([128, 512], f32, tag="avb", name="avb")
        pts = []
        for J in range(nJ + DELAY):
            if J < nJ:
                jlo = J * 128
                istart = max(jlo, qlo)
                w = qlo + 512 - istart
                pss = ps_s.tile([128, 1024], f32, tag="s", name="pss")
                for hh in range(2):
                    nc.tensor.matmul(
                        pss[:, hh * 512: hh * 512 + w],
                        lhsT=kt_sb[hh * 64:(hh + 1) * 64, p * 2048 + jlo: p * 2048 + jlo + 128],
                        rhs=qt_sb[hh * 64:(hh + 1) * 64, p * 2048 + istart: p * 2048 + istart + w],
                        start=True, stop=True)
                pt = ppt.tile([128, 1024], bf16, tag="pt", name="pt")
                nc.scalar.activation(
                    pt.rearrange("x (h c) -> x h c", c=512)[:, :, 0:w],
                    pss.rearrange("x (h c) -> x h c", c=512)[:, :, 0:w],
                    EXP, scale=SCALE)
                if jlo >= qlo:
                    # diagonal j-tile: zero the j > i triangle
                    nc.gpsimd.tensor_mul(pt[:, 0:128], pt[:, 0:128], tri_sb[:])
                    nc.gpsimd.tensor_mul(pt[:, 512:640], pt[:, 512:640], tri_sb[:])
                pts.append((pt, istart - qlo, w))
            if J == 0 and pend:
                pend.pop(0)()
            if J % 2 == 0 and filler:
                filler.pop(0)()
            Ja = J - DELAY
            if Ja < 0:
                continue
            pt, co, w = pts[Ja]
            base = Ja * VBLK + p * 130
            kw = dict(start=(Ja == 0), stop=(Ja == Jmax), skip_group_check=True)
            nc.tensor.matmul(ava[0:65, co:512], lhsT=v_sb[:, base: base + 65],
                             rhs=pt[:, 0:w], **kw)
            nc.tensor.matmul(avb[0:65, co:512], lhsT=v_sb[:, base + 65: base + 130],
                             rhs=pt[:, 512:512 + w], **kw)

        def finish():
            ones_row = onesr[64:65, :]
            rec = prec.tile([128, 1024], f32r, tag="rec", name="rec")
            with nc.allow_low_precision(reason="softmax denominators in f32r"):
                nc.vector.reciprocal(rec[64:65, 0:512], ava[64:65, :])
                nc.vector.reciprocal(rec[64:65, 512:1024], avb[64:65, :])
            # broadcast each head's reciprocal row over 64 partitions; DVE may
            # read only one PSUM operand, so stage the broadcast in SBUF.
            bc_sb = prec.tile([128, 1024], f32r, tag="bcs", name="bc_sb")
            for half in range(2):
                bc = ps_mm.tile([128, 512], f32, tag="acc", name="bc")
                nc.tensor.matmul(bc[0:64, :], lhsT=ones_row,
                                 rhs=rec[64:65, half * 512:(half + 1) * 512],
                                 start=True, stop=True)
                nc.vector.tensor_copy(bc_sb[0:64, half * 512:(half + 1) * 512],
                                      bc[0:64, :])
            nc.vector.tensor_mul(
                at_sb[0:64, p * 2048 + qlo: p * 2048 + qlo + 512],
                ava[0:64, :], bc_sb[0:64, 0:512])
            at_tmp = prec.tile([128, 512], f32r, tag="att", name="at_tmp")
            nc.vector.tensor_mul(at_tmp[0:64, :], avb[0:64, :],
                                 bc_sb[0:64, 512:1024])
            nc.sync.dma_start(
                at_sb[64:128, p * 2048 + qlo: p * 2048 + qlo + 512],
                at_tmp[0:64, :])
            if p == 1 and Q < 3:
                filler.extend(proj_stile(st) for st in range(4 * Q, 4 * Q + 4))
        pend.append(finish)

    for Q in range(4):
        att_qp(Q, 0)
        att_qp(Q, 1)
    while filler:
        filler.pop(0)()
    # tail: overlap st12's pair-0 accumulation (PSUM held open in the freed
    # score slots) with the last finisher's normalize chain, then finish the
    # remaining stiles with ACT copies while DVE/DMA drain.
    split = []
    for ec in range(2):
        ps = ps_s.tile([128, 512], f32, tag="s", name="ps_f")
        nc.tensor.matmul(ps[:], lhsT=at_sb[:, 12 * 128: 13 * 128],
                         rhs=wp_sb[:, ec * 512:(ec + 1) * 512],
                         start=True, stop=False)
        split.append(ps)
    while pend:
        pend.pop(0)()
    stage12 = pstage.tile([128, 1024], bf16, tag="stage", name="stage12")
    for ec in range(2):
        nc.tensor.matmul(split[ec][:],
                         lhsT=at_sb[:, 2048 + 12 * 128: 2048 + 13 * 128],
                         rhs=wp_sb[:, 1024 + ec * 512: 1024 + (ec + 1) * 512],
                         start=False, stop=True)
        nc.scalar.activation(stage12[:, ec * 512:(ec + 1) * 512],
                             split[ec][:], CPY)
    nc.sync.dma_start(out[12 * 128:13 * 128, :], stage12[:])
    for st in (13, 14, 15):
        proj_stile(st, on_act=True)()


def _build_nc(repeat=1):
    key = ("nc", repeat)
    if key in _CACHE:
        return _CACHE[key]
    import concourse.bacc as bacc
    import concourse.mybir as mybir
    import concourse.tile as tile

    f32 = mybir.dt.float32
    f32r = mybir.dt.float32r
    bf16d = mybir.dt.bfloat16
    nc = bacc.Bacc("TRN2", target_bir_lowering=False, debug=False)
    xt = nc.dram_tensor("xt", [D, S], bf16d, kind="ExternalInput").ap()
    wqk = nc.dram_tensor("wqk", [128, 4096], bf16d, kind="ExternalInput").ap()
    wv = nc.dram_tensor("wv", [128, 2048], bf16d, kind="ExternalInput").ap()
    wp = nc.dram_tensor("wp", [128, 2048], f32r, kind="ExternalInput").ap()
    qkb = nc.dram_tensor("qkb", [128, 4], f32, kind="ExternalInput").ap()
    vb = nc.dram_tensor("vb", [128, 256], f32, kind="ExternalInput").ap()
    tri = nc.dram_tensor("tri", [128, 128], bf16d, kind="ExternalInput").ap()
    onesd = nc.dram_tensor("onesd", [128, 64], f32r, kind="ExternalInput").ap()
    onesh = nc.dram_tensor("onesh", [128, 64], bf16d, kind="ExternalInput").ap()
    out = nc.dram_tensor("out", [S, D], bf16d, kind="ExternalOutput").ap()

    with tile.TileContext(nc) as tc:
        for _ in range(repeat):
            with ExitStack() as ctx:
                _body(ctx, tc, mybir, xt, wqk, wv, wp, qkb, vb, tri, onesd, onesh, out)
    nc.compile()
    _CACHE[key] = nc
    return nc


def _make_in_maps(hidden_states, c_attn_w, c_attn_b, c_proj_w):
    hs = np.asarray(hidden_states, dtype=np.float32)
    waw = np.asarray(c_attn_w, dtype=np.float32)
    wab = np.asarray(c_attn_b, dtype=np.float32)
    wpw = np.asarray(c_proj_w, dtype=np.float32)

    tri = np.triu(np.ones((128, 128), dtype=ml_dtypes.bfloat16))
    xts = [np.ascontiguousarray(hs[b].T).astype(ml_dtypes.bfloat16) for b in range(B)]
    in_maps = []
    for c in range(NCORES):
        b, g = divmod(c, GROUPS)
        cols = np.arange(g * HPC * HD, (g + 1) * HPC * HD)
        wqk_host = np.concatenate([waw[:, cols], waw[:, D + cols]], axis=1)
        in_maps.append({
            "xt": xts[b],
            "wqk": np.ascontiguousarray(
                wqk_host.reshape(8, 128, 512).transpose(1, 0, 2).reshape(128, 4096)).astype(ml_dtypes.bfloat16),
            "wv": np.ascontiguousarray(
                waw[:, 2 * D + cols].reshape(8, 128, 256).transpose(1, 0, 2).reshape(128, 2048)).astype(ml_dtypes.bfloat16),
            "wp": np.ascontiguousarray(
                wpw[cols, :].reshape(2, 128, 1024).transpose(1, 0, 2).reshape(128, 2048)),
            "qkb": np.ascontiguousarray(
                np.concatenate([wab[cols], wab[D + cols]]).reshape(4, 128).T),
            "vb": np.ascontiguousarray(
                np.broadcast_to(wab[2 * D + cols], (128, 256))),
            "tri": tri,
            "onesd": np.ones((128, 64), np.float32),
            "onesh": np.ones((128, 64), ml_dtypes.bfloat16),
        })
    return in_maps


def kernel(hidden_states, c_attn_w, c_attn_b, c_proj_w, c_proj_b):
    from concourse import bass_utils

    nc = _build_nc()
    in_maps = _make_in_maps(hidden_states, c_attn_w, c_attn_b, c_proj_w)
    res = bass_utils.run_bass_kernel_spmd(nc, in_maps, core_ids=list(range(NCORES)))
    outs = [np.asarray(r["out"], dtype=np.float32) for r in res.results]
    wpb = np.asarray(c_proj_b, dtype=np.float32)
    full = np.stack(
        [sum(outs[b * GROUPS:(b + 1) * GROUPS]) + wpb for b in range(B)], axis=0)
    return full.astype(np.float32)


# revision 33
# speedup vs baseline: 1.4307x; 1.0449x over previous
"""GPT-2-style causal attention block on 8 TRN2 NeuronCores (Bass/Tile).

Sharding (Megatron-style, per the hint): core c handles batch b = c // 4 and
head-group g = c % 4 (4 of the 16 heads).  Each core computes, fully locally:
  QKV projection (its 4 heads' columns), causal softmax attention for its
  4 heads, and the row-sharded output projection partial [S, D].
The host gathers by summing the 4 partials per batch and adding c_proj_b.

Per-core kernel layout choices:
  - x^T [D, S] is staged on host so Q^T/K^T come out of matmuls directly with
    head_dim on partitions (what the scores matmul wants) and V comes out in
    [seq, head_dim] (what the AV matmul wants).
  - scores are computed transposed, sT[j, i] (j = key index on partitions), so
    the exp'd tile is directly usable as the AV matmul's moving operand.
  - softmax denominator comes from the SAME matmul as AV: each head's V block
    carries a ones column ([V|1]), so psum row 64 is the rowsum and rows 0:64
    the numerator.  Normalization: DVE reciprocal of row 64, a 1-contraction
    matmul against a ones row (reused from tri) broadcasts it over 64
    partitions, then a DVE mul writes a^T.  The odd head's product lands in a
    temp tile at partitions 0:64 and is lane-shifted to at[64:128] by a tiny
    SBUF->SBUF DMA.  This halves attention PE work vs a separate ones-matmul.
  - QKV phase A runs k-outer while x^T/W stream from HBM: Q/K (i-halves
    sc0, sc1) and V (j0-3) accumulate in 8 live PSUM banks, so PE tracks the
    input DMA instead of idling.  x^T lands in 512-col quarters for finer
    pipelining.  The rest of QKV + the output projection are emitted as
    "filler" work units between attention J-steps, keeping PE busy while the
    exp stream paces softmax.
  - x/Wqkv/scores/probs/V run in bf16; the output projection in float32r.
"""

from contextlib import ExitStack

import ml_dtypes
import numpy as np

B, S, D = 2, 2048, 1024
NH, HD = 16, 64
NCORES = 8
GROUPS = 4           # tensor-parallel head groups per batch
HPC = NH // GROUPS   # heads per core
SCALE = 1.0 / 8.0    # 1/sqrt(HD)
VBLK = 260           # per-j-tile V block: [V0|1|V1|1|V2|1|V3|1]

_CACHE = {}


def _body(ctx, tc, mybir, xt, wqk, wv, wp, wp2, qkb, vb, tri, onesd, onesh, out):
    nc = tc.nc
    f32 = mybir.dt.float32
    f32r = mybir.dt.float32r
    bf16 = mybir.dt.bfloat16
    EXP = mybir.ActivationFunctionType.Exp

    pin = ctx.enter_context(tc.tile_pool(name="pin", bufs=1))
    pwork = ctx.enter_context(tc.tile_pool(name="pwork", bufs=1))
    ppt = ctx.enter_context(tc.tile_pool(name="ppt", bufs=10))
    prec = ctx.enter_context(tc.tile_pool(name="prec", bufs=4))
    pstage = ctx.enter_context(tc.tile_pool(name="pstage", bufs=4))
    ps_mm = ctx.enter_context(tc.tile_pool(name="ps_mm", bufs=2, space="PSUM"))
    ps_s = ctx.enter_context(tc.tile_pool(name="ps_s", bufs=2, space="PSUM"))
    ps_av = ctx.enter_context(tc.tile_pool(name="ps_av", bufs=1, space="PSUM"))

    # ---------------- input staging ----------------
    # Each DMA costs ~625ns of serial HWDGE time on top of its transfer, so
    # batch inputs into few, large DMAs ordered by first use: the first wqk
    # block + x^T chunk 0 unblock the first phase-A matmul ASAP, small
    # constants slot into the stream just before their first consumer.
    xt_sb = pin.tile([128, 8 * 2048], bf16, name="xt_sb")
    wqk_sb = pin.tile([128, 4096], bf16, name="wqk_sb")
    wv_sb = pin.tile([128, 2048], bf16, name="wv_sb")
    nc.sync.dma_start(wqk_sb[:, 0:512], wqk[:, 0:512])
    nc.sync.dma_start(xt_sb[:, 0:2048], xt[0:128, :])
    nc.sync.dma_start(wv_sb[:], wv[:])
    nc.sync.dma_start(wqk_sb[:, 512:4096], wqk[:, 512:4096])
    for k in range(1, 5):
        nc.sync.dma_start(xt_sb[:, k * 2048:(k + 1) * 2048],
                          xt[k * 128:(k + 1) * 128, :])
    qkb_sb = pin.tile([128, 4], f32, name="qkb_sb")
    nc.sync.dma_start(qkb_sb[:], qkb[:])
    tri_sb = pin.tile([128, 128], bf16, name="tri_sb")
    nc.sync.dma_start(tri_sb[:], tri[:])
    for k in range(5, 8):
        nc.sync.dma_start(xt_sb[:, k * 2048:(k + 1) * 2048],
                          xt[k * 128:(k + 1) * 128, :])
    vb_sb = pin.tile([128, 256], f32, name="vb_sb")
    nc.sync.dma_start(vb_sb[:], vb[:])
    onesr = pin.tile([128, 64], f32r, name="onesr")
    nc.sync.dma_start(onesr[:], onesd[:])
    ones64 = pin.tile([128, 64], bf16, name="ones64")
    nc.sync.dma_start(ones64[:], onesh[:])
    wp_sb = pin.tile([128, 2048], f32r, name="wp_sb")
    nc.sync.dma_start(wp_sb[:], wp[:])
    wp2_sb = pin.tile([128, 1024], f32r, name="wp2_sb")
    nc.sync.dma_start(wp2_sb[:], wp2[:])

    # Q^T / K^T: head-pair p at cols [p*2048, (p+1)*2048); head hh of the pair
    # on partitions [hh*64, hh*64+64).
    qt_sb = pwork.tile([128, 2 * 2048], bf16, name="qt_sb")
    kt_sb = pwork.tile([128, 2 * 2048], bf16, name="kt_sb")
    # V blocks of VBLK cols per j-tile: head h's [V_h|1] at cols h*65
    v_sb = pwork.tile([128, 16 * VBLK], bf16, name="v_sb")
    # a^T: k2 (head pair) at cols [k2*2048, ...), head hh on partitions hh*64..
    at_sb = pwork.tile([128, 2 * 2048], f32r, name="at_sb")

    # Dummy exp so the ACT table set loads during the input-DMA window instead
    # of delaying the first real softmax exp.
    warm = pin.tile([128, 4], f32, name="warm")
    nc.scalar.activation(warm[:], qkb_sb[:], EXP, scale=0.0)

    # ---------------- QKV helpers ----------------
    # col-tiles: C=0 -> Q pair0, C=1 -> Q pair1, C=2 -> K pair0, C=3 -> K pair1
    # psum->SBUF bias-add moves run on ACT (Copy is in the exp table set, and
    # ACT is idle during the QKV phases) to keep DVE free for softmax work.
    CPY = mybir.ActivationFunctionType.Copy
    IDN = mybir.ActivationFunctionType.Identity

    def qk_add(ps_ap, C, sc):
        dest = qt_sb if C < 2 else kt_sb
        p = C % 2
        nc.scalar.activation(
            dest[:, p * 2048 + sc * 512: p * 2048 + (sc + 1) * 512],
            ps_ap, IDN, bias=qkb_sb[:, C:C + 1])

    def v_add(ps_ap256, j):
        dst = v_sb[:, j * VBLK:(j + 1) * VBLK].rearrange(
            "p (g c) -> p g c", c=65)[:, :, 0:64]
        nc.vector.tensor_add(
            dst,
            ps_ap256.rearrange("p (g c) -> p g c", c=64),
            vb_sb.rearrange("p (g c) -> p g c", c=64))

    # ---------------- phase A: k-outer QKV subset ----------------
    # 8 live PSUM banks track the input stream: Q/K for sc0 (acc0/acc1),
    # sc1 (ava/avb), V j0-3 packed into one 2-bank tile (s).
    a_q0 = ps_mm.tile([128, 512], f32, tag="acc", name="a_q0")
    a_k0 = ps_mm.tile([128, 512], f32, tag="acc", name="a_k0")
    a_q1 = ps_av.tile([128, 512], f32, tag="ava", name="a_q1")
    a_k1 = ps_av.tile([128, 512], f32, tag="avb", name="a_k1")
    a_v = ps_s.tile([128, 1024], f32, tag="s", name="a_v")
    for k in range(8):
        for (ps_t, C, sc) in ((a_q0, 0, 0), (a_k0, 2, 0)):
            nc.tensor.matmul(
                ps_t[:],
                lhsT=wqk_sb[:, k * 512 + C * 128: k * 512 + (C + 1) * 128],
                rhs=xt_sb[:, k * 2048 + sc * 512: k * 2048 + (sc + 1) * 512],
                start=(k == 0), stop=(k == 7))
        for j in range(4):
            # start=True zeroes the whole 2KB bank region, so only the first
            # group per bank (j=0 for cols 0:512, j=2 for 512:1024) may start.
            nc.tensor.matmul(
                a_v[:, j * 256:(j + 1) * 256],
                lhsT=xt_sb[:, k * 2048 + j * 128: k * 2048 + (j + 1) * 128],
                rhs=wv_sb[:, k * 256:(k + 1) * 256],
                start=(k == 0 and j % 2 == 0), stop=(k == 7),
                skip_group_check=True)
        for (ps_t, C, sc) in ((a_q1, 0, 1), (a_k1, 2, 1)):
            nc.tensor.matmul(
                ps_t[:],
                lhsT=wqk_sb[:, k * 512 + C * 128: k * 512 + (C + 1) * 128],
                rhs=xt_sb[:, k * 2048 + sc * 512: k * 2048 + (sc + 1) * 512],
                start=(k == 0), stop=(k == 7))
    qk_add(a_k0[:], 2, 0)
    qk_add(a_q0[:], 0, 0)
    for j in range(4):
        v_add(a_v[:, j * 256:(j + 1) * 256], j)
    qk_add(a_q1[:], 0, 1)
    qk_add(a_k1[:], 2, 1)
    # ones columns of the V blocks (emitted after the v_adds so the DVE queue
    # is not head-blocked waiting for the onesh DMA; disjoint columns)
    nc.vector.tensor_copy(
        v_sb.rearrange("p (g c) -> p g c", c=65)[:, :, 64:65],
        ones64.rearrange("p (g c) -> p g c", c=1))

    # ---------------- filler work units ----------------
    def qk_block(sc, C):
        def go():
            ps = ps_mm.tile([128, 512], f32, tag="acc", name="qkB")
            for k in range(8):
                nc.tensor.matmul(
                    ps[:],
                    lhsT=wqk_sb[:, k * 512 + C * 128: k * 512 + (C + 1) * 128],
                    rhs=xt_sb[:, k * 2048 + sc * 512: k * 2048 + (sc + 1) * 512],
                    start=(k == 0), stop=(k == 7))
            qk_add(ps[:], C, sc)
        return go

    def v_block(j):
        def go():
            ps = ps_mm.tile([128, 256], f32, tag="acc", name="vB")
            for k in range(8):
                nc.tensor.matmul(
                    ps[:],
                    lhsT=xt_sb[:, k * 2048 + j * 128: k * 2048 + (j + 1) * 128],
                    rhs=wv_sb[:, k * 256:(k + 1) * 256],
                    start=(k == 0), stop=(k == 7))
            v_add(ps[:], j)
        return go

    def proj_stile(st, on_act=False):
        def go():
            stage = pstage.tile([128, 1024], bf16, tag="stage", name="stage")
            for ec in range(2):
                ps = ps_mm.tile([128, 512], f32, tag="acc", name="ps_o")
                for k2 in range(2):
                    nc.tensor.matmul(
                        ps[:],
                        lhsT=at_sb[:, k2 * 2048 + st * 128: k2 * 2048 + (st + 1) * 128],
                        rhs=wp_sb[:, k2 * 1024 + ec * 512: k2 * 1024 + (ec + 1) * 512],
                        start=(k2 == 0), stop=(k2 == 1))
                if on_act:  # tail stiles: ACT is idle once the exps are done
                    nc.scalar.activation(stage[:, ec * 512:(ec + 1) * 512],
                                         ps[:], CPY)
                else:
                    nc.vector.tensor_copy(stage[:, ec * 512:(ec + 1) * 512],
                                          ps[:])
            nc.sync.dma_start(out[st * 128:(st + 1) * 128, :], stage[:])
        return go

    filler = [qk_block(0, 1), qk_block(0, 3),
              qk_block(2, 0), qk_block(2, 2),
              qk_block(3, 0), qk_block(3, 2),
              qk_block(1, 1), qk_block(1, 3),
              v_block(4), v_block(5), v_block(6), v_block(7),
              qk_block(2, 1), qk_block(2, 3),
              v_block(8), v_block(9), v_block(10), v_block(11),
              qk_block(3, 1), qk_block(3, 3),
              v_block(12), v_block(13), v_block(14), v_block(15)]
    pend = []  # deferred attention finishers (normalize + a^T write)
    last_at = []  # (3,1)'s odd-head a^T temp, consumed by the tail

    # ---------------- attention ----------------
    # Per (i-quarter Q of 512, head-pair p).  Scores for both heads of the
    # pair share one [128, 1024] PSUM tile (head hh at cols hh*512) so one
    # strided exp covers both.  AV psums (per head):
    #   av[0:65] = [V|1]^T probs  -> rows 0:64 numerator, row 64 rowsum
    # The finisher normalizes lane-aligned at partitions 0:64 and lane-shifts
    # the odd head's a^T to partitions 64:128 with a SBUF->SBUF DMA.
    DELAY = 3  # software-pipeline distance between scores/exp and AV use

    def att_qp(Q, p):
        qlo = Q * 512
        Jmax = 4 * Q + 3
        nJ = 4 * Q + 4
        ava = ps_av.tile([128, 512], f32, tag="ava", name="ava")
        avb = ps_av.tile([128, 512], f32, tag="avb", name="avb")
        pts = []
        for J in range(nJ + DELAY):
            if J < nJ:
                jlo = J * 128
                istart = max(jlo, qlo)
                w = qlo + 512 - istart
                pss = ps_s.tile([128, 1024], f32, tag="s", name="pss")
                for hh in range(2):
                    nc.tensor.matmul(
                        pss[:, hh * 512: hh * 512 + w],
                        lhsT=kt_sb[hh * 64:(hh + 1) * 64, p * 2048 + jlo: p * 2048 + jlo + 128],
                        rhs=qt_sb[hh * 64:(hh + 1) * 64, p * 2048 + istart: p * 2048 + istart + w],
                        start=True, stop=True)
                pt = ppt.tile([128, 1024], bf16, tag="pt", name="pt")
                nc.scalar.activation(
                    pt.rearrange("x (h c) -> x h c", c=512)[:, :, 0:w],
                    pss.rearrange("x (h c) -> x h c", c=512)[:, :, 0:w],
                    EXP, scale=SCALE)
                if jlo >= qlo:
                    # diagonal j-tile: zero the j > i triangle
                    nc.gpsimd.tensor_mul(pt[:, 0:128], pt[:, 0:128], tri_sb[:])
                    nc.gpsimd.tensor_mul(pt[:, 512:640], pt[:, 512:640], tri_sb[:])
                pts.append((pt, istart - qlo, w))
            if J == 0 and pend:
                pend.pop(0)()
            if J % 2 == 0 and filler:
                filler.pop(0)()
            Ja = J - DELAY
            if Ja < 0:
                continue
            pt, co, w = pts[Ja]
            base = Ja * VBLK + p * 130
            kw = dict(start=(Ja == 0), stop=(Ja == Jmax), skip_group_check=True)
            nc.tensor.matmul(ava[0:65, co:512], lhsT=v_sb[:, base: base + 65],
                             rhs=pt[:, 0:w], **kw)
            nc.tensor.matmul(avb[0:65, co:512], lhsT=v_sb[:, base + 65: base + 130],
                             rhs=pt[:, 512:512 + w], **kw)

        def finish():
            ones_row = onesr[64:65, :]
            rec = prec.tile([128, 1024], f32r, tag="rec", name="rec")
            with nc.allow_low_precision(reason="softmax denominators in f32r"):
                nc.vector.reciprocal(rec[64:65, 0:512], ava[64:65, :])
                nc.vector.reciprocal(rec[64:65, 512:1024], avb[64:65, :])
            # broadcast each head's reciprocal row over 64 partitions; DVE may
            # read only one PSUM operand, so stage the broadcast in SBUF.
            bc_sb = prec.tile([128, 1024], f32r, tag="bcs", name="bc_sb")
            for half in range(2):
                bc = ps_mm.tile([128, 512], f32, tag="acc", name="bc")
                nc.tensor.matmul(bc[0:64, :], lhsT=ones_row,
                                 rhs=rec[64:65, half * 512:(half + 1) * 512],
                                 start=True, stop=True)
                nc.vector.tensor_copy(bc_sb[0:64, half * 512:(half + 1) * 512],
                                      bc[0:64, :])
            nc.vector.tensor_mul(
                at_sb[0:64, p * 2048 + qlo: p * 2048 + qlo + 512],
                ava[0:64, :], bc_sb[0:64, 0:512])
            at_tmp = prec.tile([128, 512], f32r, tag="att", name="at_tmp")
            nc.vector.tensor_mul(at_tmp[0:64, :], avb[0:64, :],
                                 bc_sb[0:64, 512:1024])
            if Q == 3 and p == 1:
                # the tail reads this tile directly (split contraction), so
                # the lane-shift DMA is off the closing critical path
                last_at.append(at_tmp)
            else:
                nc.sync.dma_start(
                    at_sb[64:128, p * 2048 + qlo: p * 2048 + qlo + 512],
                    at_tmp[0:64, :])
            if p == 1 and Q < 3:
                filler.extend(proj_stile(st) for st in range(4 * Q, 4 * Q + 4))
        pend.append(finish)

    for Q in range(4):
        att_qp(Q, 0)
        att_qp(Q, 1)
    while filler:
        filler.pop(0)()
    # tail: overlap st12's pair-0 accumulation (PSUM held open in the freed
    # score slots) with the last finisher's normalize chain, then finish the
    # remaining stiles with ACT copies while DVE/DMA drain.
    split = []
    for ec in range(2):
        ps = ps_s.tile([128, 512], f32, tag="s", name="ps_f")
        nc.tensor.matmul(ps[:], lhsT=at_sb[:, 12 * 128: 13 * 128],
                         rhs=wp_sb[:, ec * 512:(ec + 1) * 512],
                         start=True, stop=False)
        split.append(ps)
    while pend:
        pend.pop(0)()
    att3 = last_at[0]

    def tail_k2(ps, st, ec):
        # pair-1 contraction split: even head from at_sb[0:64], odd head
        # straight from the finisher's temp tile (no lane-shift DMA)
        nc.tensor.matmul(
            ps[:], lhsT=at_sb[0:64, 2048 + st * 128: 2048 + (st + 1) * 128],
            rhs=wp_sb[0:64, 1024 + ec * 512: 1024 + (ec + 1) * 512],
            start=False, stop=False)
        nc.tensor.matmul(
            ps[:], lhsT=att3[0:64, (st - 12) * 128: (st - 11) * 128],
            rhs=wp2_sb[0:64, ec * 512:(ec + 1) * 512],
            start=False, stop=True)

    stage12 = pstage.tile([128, 1024], bf16, tag="stage", name="stage12")
    for ec in range(2):
        tail_k2(split[ec], 12, ec)
        nc.scalar.activation(stage12[:, ec * 512:(ec + 1) * 512],
                             split[ec][:], CPY)
    nc.sync.dma_start(out[12 * 128:13 * 128, :], stage12[:])
    for st in (13, 14, 15):
        stage = pstage.tile([128, 1024], bf16, tag="stage", name="stageT")
        for ec in range(2):
            ps = ps_mm.tile([128, 512], f32, tag="acc", name="ps_t")
            nc.tensor.matmul(ps[:], lhsT=at_sb[:, st * 128:(st + 1) * 128],
                             rhs=wp_sb[:, ec * 512:(ec + 1) * 512],
                             start=True, stop=False)
            tail_k2(ps, st, ec)
            nc.scalar.activation(stage[:, ec * 512:(ec + 1) * 512], ps[:], CPY)
        nc.sync.dma_start(out[st * 128:(st + 1) * 128, :], stage[:])


def _build_nc(repeat=1):
    key = ("nc", repeat)
    if key in _CACHE:
        return _CACHE[key]
    import concourse.bacc as bacc
    import concourse.mybir as mybir
    import concourse.tile as tile

    f32 = mybir.dt.float32
    f32r = mybir.dt.float32r
    bf16d = mybir.dt.bfloat16
    nc = bacc.Bacc("TRN2", target_bir_lowering=False, debug=False)
    xt = nc.dram_tensor("xt", [D, S], bf16d, kind="ExternalInput").ap()
    wqk = nc.dram_tensor("wqk", [128, 4096], bf16d, kind="ExternalInput").ap()
    wv = nc.dram_tensor("wv", [128, 2048], bf16d, kind="ExternalInput").ap()
    wp = nc.dram_tensor("wp", [128, 2048], f32r, kind="ExternalInput").ap()
    wp2 = nc.dram_tensor("wp2", [128, 1024], f32r, kind="ExternalInput").ap()
    qkb = nc.dram_tensor("qkb", [128, 4], f32, kind="ExternalInput").ap()
    vb = nc.dram_tensor("vb", [128, 256], f32, kind="ExternalInput").ap()
    tri = nc.dram_tensor("tri", [128, 128], bf16d, kind="ExternalInput").ap()
    onesd = nc.dram_tensor("onesd", [128, 64], f32r, kind="ExternalInput").ap()
    onesh = nc.dram_tensor("onesh", [128, 64], bf16d, kind="ExternalInput").ap()
    out = nc.dram_tensor("out", [S, D], bf16d, kind="ExternalOutput").ap()

    with tile.TileContext(nc) as tc:
        for _ in range(repeat):
            with ExitStack() as ctx:
                _body(ctx, tc, mybir, xt, wqk, wv, wp, wp2, qkb, vb, tri, onesd, onesh, out)
    nc.compile()
    _CACHE[key] = nc
    return nc


def _make_in_maps(hidden_states, c_attn_w, c_attn_b, c_proj_w):
    hs = np.asarray(hidden_states, dtype=np.float32)
    waw = np.asarray(c_attn_w, dtype=np.float32)
    wab = np.asarray(c_attn_b, dtype=np.float32)
    wpw = np.asarray(c_proj_w, dtype=np.float32)

    tri = np.triu(np.ones((128, 128), dtype=ml_dtypes.bfloat16))
    xts = [np.ascontiguousarray(hs[b].T).astype(ml_dtypes.bfloat16) for b in range(B)]
    in_maps = []
    for c in range(NCORES):
        b, g = divmod(c, GROUPS)
        cols = np.arange(g * HPC * HD, (g + 1) * HPC * HD)
        wqk_host = np.concatenate([waw[:, cols], waw[:, D + cols]], axis=1)
        in_maps.append({
            "xt": xts[b],
            "wqk": np.ascontiguousarray(
                wqk_host.reshape(8, 128, 512).transpose(1, 0, 2).reshape(128, 4096)).astype(ml_dtypes.bfloat16),
            "wv": np.ascontiguousarray(
                waw[:, 2 * D + cols].reshape(8, 128, 256).transpose(1, 0, 2).reshape(128, 2048)).astype(ml_dtypes.bfloat16),
            "wp": np.ascontiguousarray(
                wpw[cols, :].reshape(2, 128, 1024).transpose(1, 0, 2).reshape(128, 2048)),
            "wp2": np.concatenate(
                [wpw[cols[192:256], :], np.zeros((64, D), np.float32)], axis=0),
            "qkb": np.ascontiguousarray(
                np.concatenate([wab[cols], wab[D + cols]]).reshape(4, 128).T),
            "vb": np.ascontiguousarray(
                np.broadcast_to(wab[2 * D + cols], (128, 256))),
            "tri": tri,
            "onesd": np.ones((128, 64), np.float32),
            "onesh": np.ones((128, 64), ml_dtypes.bfloat16),
        })
    return in_maps


def kernel(hidden_states, c_attn_w, c_attn_b, c_proj_w, c_proj_b):
    from concourse import bass_utils

    nc = _build_nc()
    in_maps = _make_in_maps(hidden_states, c_attn_w, c_attn_b, c_proj_w)
    res = bass_utils.run_bass_kernel_spmd(nc, in_maps, core_ids=list(range(NCORES)))
    outs = [np.asarray(r["out"], dtype=np.float32) for r in res.results]
    wpb = np.asarray(c_proj_b, dtype=np.float32)
    full = np.stack(
        [sum(outs[b * GROUPS:(b + 1) * GROUPS]) + wpb for b in range(B)], axis=0)
    return full.astype(np.float32)


# revision 57
# speedup vs baseline: 1.4827x; 1.0363x over previous
"""GPT-2-style causal attention block on 8 TRN2 NeuronCores (Bass/Tile).

Sharding (Megatron-style, per the hint): core c handles batch b = c // 4 and
head-group g = c % 4 (4 of the 16 heads).  Each core computes, fully locally:
  QKV projection (its 4 heads' columns), causal softmax attention for its
  4 heads, and the row-sharded output projection partial [S, D].
The host gathers by summing the 4 partials per batch and adding c_proj_b.

Per-core kernel layout choices:
  - x^T [D, S] is staged on host so Q^T/K^T come out of matmuls directly with
    head_dim on partitions (what the scores matmul wants) and V comes out in
    [seq, head_dim] (what the AV matmul wants).
  - scores are computed transposed, sT[j, i] (j = key index on partitions), so
    the exp'd tile is directly usable as the AV matmul's moving operand.
  - softmax denominator comes from the SAME matmul as AV: each head's V block
    carries a ones column ([V|1]), so psum row 64 is the rowsum and rows 0:64
    the numerator.  Normalization: DVE reciprocal of row 64, a 1-contraction
    matmul against a ones row (reused from tri) broadcasts it over 64
    partitions, then a DVE mul writes a^T.  The odd head's product lands in a
    temp tile at partitions 0:64 and is lane-shifted to at[64:128] by a tiny
    SBUF->SBUF DMA.  This halves attention PE work vs a separate ones-matmul.
  - QKV phase A runs k-outer while x^T/W stream from HBM: Q/K (i-halves
    sc0, sc1) and V (j0-3) accumulate in 8 live PSUM banks, so PE tracks the
    input DMA instead of idling.  x^T lands in 512-col quarters for finer
    pipelining.  The rest of QKV + the output projection are emitted as
    "filler" work units between attention J-steps, keeping PE busy while the
    exp stream paces softmax.
  - x/Wqkv/scores/probs/V run in bf16; the output projection in float32r.
"""

from contextlib import ExitStack

import ml_dtypes
import numpy as np

B, S, D = 2, 2048, 1024
NH, HD = 16, 64
NCORES = 8
GROUPS = 4           # tensor-parallel head groups per batch
HPC = NH // GROUPS   # heads per core
SCALE = 1.0 / 8.0    # 1/sqrt(HD)
VBLK = 260           # per-j-tile V block: [V0|1|V1|1|V2|1|V3|1]

_CACHE = {}


def _body(ctx, tc, mybir, xt, wqk, wv, wp, wp2, qkb, vb, tri, onesd, onesh, out):
    nc = tc.nc
    f32 = mybir.dt.float32
    f32r = mybir.dt.float32r
    bf16 = mybir.dt.bfloat16
    EXP = mybir.ActivationFunctionType.Exp

    pin = ctx.enter_context(tc.tile_pool(name="pin", bufs=1))
    pwork = ctx.enter_context(tc.tile_pool(name="pwork", bufs=1))
    ppt = ctx.enter_context(tc.tile_pool(name="ppt", bufs=10))
    prec = ctx.enter_context(tc.tile_pool(name="prec", bufs=4))
    pstage = ctx.enter_context(tc.tile_pool(name="pstage", bufs=4))
    ps_mm = ctx.enter_context(tc.tile_pool(name="ps_mm", bufs=2, space="PSUM"))
    ps_s = ctx.enter_context(tc.tile_pool(name="ps_s", bufs=2, space="PSUM"))
    ps_av = ctx.enter_context(tc.tile_pool(name="ps_av", bufs=1, space="PSUM"))

    # ---------------- input staging ----------------
    # Each DMA costs ~625ns of serial HWDGE time on top of its transfer, so
    # batch inputs into few, large DMAs ordered by first use: the first wqk
    # block + x^T chunk 0 unblock the first phase-A matmul ASAP, small
    # constants slot into the stream just before their first consumer.
    xt_sb = pin.tile([128, 8 * 2048], bf16, name="xt_sb")
    wqk_sb = pin.tile([128, 4096], bf16, name="wqk_sb")
    wv_sb = pin.tile([128, 2048], bf16, name="wv_sb")
    nc.sync.dma_start(wqk_sb[:, 0:512], wqk[:, 0:512])
    nc.sync.dma_start(xt_sb[:, 0:512], xt[0:128, 0:512])
    nc.sync.dma_start(wv_sb[:], wv[:])
    nc.sync.dma_start(xt_sb[:, 512:2048], xt[0:128, 512:2048])
    for k in range(1, 8):
        nc.sync.dma_start(wqk_sb[:, k * 512:(k + 1) * 512],
                          wqk[:, k * 512:(k + 1) * 512])
        nc.sync.dma_start(xt_sb[:, k * 2048:(k + 1) * 2048],
                          xt[k * 128:(k + 1) * 128, :])
    qkb_sb = pin.tile([128, 4], f32, name="qkb_sb")
    nc.sync.dma_start(qkb_sb[:], qkb[:])
    tri_sb = pin.tile([128, 128], bf16, name="tri_sb")
    nc.sync.dma_start(tri_sb[:], tri[:])
    vb_sb = pin.tile([128, 256], f32, name="vb_sb")
    nc.sync.dma_start(vb_sb[:], vb[:])
    onesr = pin.tile([128, 64], f32r, name="onesr")
    nc.sync.dma_start(onesr[:], onesd[:])
    ones64 = pin.tile([128, 64], bf16, name="ones64")
    nc.sync.dma_start(ones64[:], onesh[:])
    wp_sb = pin.tile([128, 2048], f32r, name="wp_sb")
    nc.sync.dma_start(wp_sb[:], wp[:])
    wp2_sb = pin.tile([128, 1024], f32r, name="wp2_sb")
    nc.sync.dma_start(wp2_sb[:], wp2[:])

    # Q^T / K^T: head-pair p at cols [p*2048, (p+1)*2048); head hh of the pair
    # on partitions [hh*64, hh*64+64).
    qt_sb = pwork.tile([128, 2 * 2048], bf16, name="qt_sb")
    kt_sb = pwork.tile([128, 2 * 2048], bf16, name="kt_sb")
    # V blocks of VBLK cols per j-tile: head h's [V_h|1] at cols h*65
    v_sb = pwork.tile([128, 16 * VBLK], bf16, name="v_sb")
    # a^T: k2 (head pair) at cols [k2*2048, ...), head hh on partitions hh*64..
    at_sb = pwork.tile([128, 2 * 2048], f32r, name="at_sb")

    # Dummy exp so the ACT table set loads during the input-DMA window instead
    # of delaying the first real softmax exp (wv lands within ~5us).
    warm = pin.tile([128, 4], f32, name="warm")
    nc.scalar.activation(warm[:], wv_sb[:, 0:4], EXP, scale=0.0)

    # ---------------- QKV helpers ----------------
    # col-tiles: C=0 -> Q pair0, C=1 -> Q pair1, C=2 -> K pair0, C=3 -> K pair1
    # psum->SBUF bias-add moves run on ACT (Copy is in the exp table set, and
    # ACT is idle during the QKV phases) to keep DVE free for softmax work.
    CPY = mybir.ActivationFunctionType.Copy
    IDN = mybir.ActivationFunctionType.Identity

    def qk_add(ps_ap, C, sc, on_act=True):
        dest = qt_sb if C < 2 else kt_sb
        p = C % 2
        dst = dest[:, p * 2048 + sc * 512: p * 2048 + (sc + 1) * 512]
        if on_act:
            nc.scalar.activation(dst, ps_ap, IDN, bias=qkb_sb[:, C:C + 1])
        else:  # late blocks run inside ACT-paced attention windows
            nc.vector.tensor_scalar_add(dst, ps_ap, qkb_sb[:, C:C + 1])

    def v_add(ps_ap256, j):
        dst = v_sb[:, j * VBLK:(j + 1) * VBLK].rearrange(
            "p (g c) -> p g c", c=65)[:, :, 0:64]
        nc.vector.tensor_add(
            dst,
            ps_ap256.rearrange("p (g c) -> p g c", c=64),
            vb_sb.rearrange("p (g c) -> p g c", c=64))

    # ---------------- phase A: k-outer QKV subset ----------------
    # All 8 PSUM banks track the input stream: Q/K for sc0 (acc0/acc1),
    # sc1 (ava/avb), sc2 (packed 2-bank s tile), V j0-3 (packed s tile).
    a_q0 = ps_mm.tile([128, 512], f32, tag="acc", name="a_q0")
    a_k0 = ps_mm.tile([128, 512], f32, tag="acc", name="a_k0")
    a_q1 = ps_av.tile([128, 512], f32, tag="ava", name="a_q1")
    a_k1 = ps_av.tile([128, 512], f32, tag="avb", name="a_k1")
    a_v = ps_s.tile([128, 1024], f32, tag="s", name="a_v")
    a_qk2 = ps_s.tile([128, 1024], f32, tag="s", name="a_qk2")
    for k in range(8):
        for (ps_t, C, sc) in ((a_q0, 0, 0), (a_k0, 2, 0)):
            nc.tensor.matmul(
                ps_t[:],
                lhsT=wqk_sb[:, k * 512 + C * 128: k * 512 + (C + 1) * 128],
                rhs=xt_sb[:, k * 2048 + sc * 512: k * 2048 + (sc + 1) * 512],
                start=(k == 0), stop=(k == 7))
        for j in range(4):
            # start=True zeroes the whole 2KB bank region, so only the first
            # group per bank (j=0 for cols 0:512, j=2 for 512:1024) may start.
            nc.tensor.matmul(
                a_v[:, j * 256:(j + 1) * 256],
                lhsT=xt_sb[:, k * 2048 + j * 128: k * 2048 + (j + 1) * 128],
                rhs=wv_sb[:, k * 256:(k + 1) * 256],
                start=(k == 0 and j % 2 == 0), stop=(k == 7),
                skip_group_check=True)
        for (ps_t, C, sc) in ((a_q1, 0, 1), (a_k1, 2, 1)):
            nc.tensor.matmul(
                ps_t[:],
                lhsT=wqk_sb[:, k * 512 + C * 128: k * 512 + (C + 1) * 128],
                rhs=xt_sb[:, k * 2048 + sc * 512: k * 2048 + (sc + 1) * 512],
                start=(k == 0), stop=(k == 7))
        for (co, C) in ((0, 0), (512, 2)):  # sc2, one group per bank
            nc.tensor.matmul(
                a_qk2[:, co:co + 512],
                lhsT=wqk_sb[:, k * 512 + C * 128: k * 512 + (C + 1) * 128],
                rhs=xt_sb[:, k * 2048 + 1024: k * 2048 + 1536],
                start=(k == 0), stop=(k == 7))
    qk_add(a_k0[:], 2, 0)
    qk_add(a_q0[:], 0, 0)
    for j in range(4):
        v_add(a_v[:, j * 256:(j + 1) * 256], j)
    qk_add(a_q1[:], 0, 1)
    qk_add(a_k1[:], 2, 1)
    qk_add(a_qk2[:, 0:512], 0, 2)
    qk_add(a_qk2[:, 512:1024], 2, 2)
    # ones columns of the V blocks (emitted after the v_adds so the DVE queue
    # is not head-blocked waiting for the onesh DMA; disjoint columns)
    nc.vector.tensor_copy(
        v_sb.rearrange("p (g c) -> p g c", c=65)[:, :, 64:65],
        ones64.rearrange("p (g c) -> p g c", c=1))

    # ---------------- filler work units ----------------
    def qk_block(sc, C, on_act=True):
        def go():
            ps = ps_mm.tile([128, 512], f32, tag="acc", name="qkB")
            for k in range(8):
                nc.tensor.matmul(
                    ps[:],
                    lhsT=wqk_sb[:, k * 512 + C * 128: k * 512 + (C + 1) * 128],
                    rhs=xt_sb[:, k * 2048 + sc * 512: k * 2048 + (sc + 1) * 512],
                    start=(k == 0), stop=(k == 7))
            qk_add(ps[:], C, sc, on_act=on_act)
        return go

    def v_block(j):
        def go():
            ps = ps_mm.tile([128, 256], f32, tag="acc", name="vB")
            for k in range(8):
                nc.tensor.matmul(
                    ps[:],
                    lhsT=xt_sb[:, k * 2048 + j * 128: k * 2048 + (j + 1) * 128],
                    rhs=wv_sb[:, k * 256:(k + 1) * 256],
                    start=(k == 0), stop=(k == 7))
            v_add(ps[:], j)
        return go

    def proj_stile(st, on_act=False):
        def go():
            stage = pstage.tile([128, 1024], bf16, tag="stage", name="stage")
            for ec in range(2):
                ps = ps_mm.tile([128, 512], f32, tag="acc", name="ps_o")
                for k2 in range(2):
                    nc.tensor.matmul(
                        ps[:],
                        lhsT=at_sb[:, k2 * 2048 + st * 128: k2 * 2048 + (st + 1) * 128],
                        rhs=wp_sb[:, k2 * 1024 + ec * 512: k2 * 1024 + (ec + 1) * 512],
                        start=(k2 == 0), stop=(k2 == 1))
                if on_act:  # tail stiles: ACT is idle once the exps are done
                    nc.scalar.activation(stage[:, ec * 512:(ec + 1) * 512],
                                         ps[:], CPY)
                else:
                    nc.vector.tensor_copy(stage[:, ec * 512:(ec + 1) * 512],
                                          ps[:])
            nc.sync.dma_start(out[st * 128:(st + 1) * 128, :], stage[:])
        return go

    # Filler queue: (key, closure) where key = 2*Q + p of the earliest
    # attention instance allowed to pop it.  Reserving late work for the
    # ACT-saturated Q2/Q3 stretches keeps PE fed end-to-end.
    filler = [(0, qk_block(0, 1)), (0, qk_block(0, 3)),
              (0, qk_block(1, 1)), (0, qk_block(1, 3)),
              (1, v_block(4)), (1, v_block(5)),
              (1, v_block(6)), (1, v_block(7)),
              (2, qk_block(3, 0, False)), (2, qk_block(3, 2, False)),
              (2, v_block(8)), (2, v_block(9)),
              (3, v_block(10)), (3, v_block(11)),
              (4, qk_block(2, 1, False)), (4, qk_block(2, 3, False)),
              (6, qk_block(3, 1, False)), (6, qk_block(3, 3, False)),
              (6, v_block(12)), (6, v_block(13)),
              (6, v_block(14)), (6, v_block(15))]

    def pop_filler(key):
        for i, (mk, go) in enumerate(filler):
            if mk <= key:
                filler.pop(i)
                go()
                return

    pend = []  # deferred attention finishers (normalize + a^T write)
    last_at = []  # (3,1)'s odd-head a^T temp, consumed by the tail

    # ---------------- attention ----------------
    # Per (i-quarter Q of 512, head-pair p).  Scores for both heads of the
    # pair share one [128, 1024] PSUM tile (head hh at cols hh*512) so one
    # strided exp covers both.  AV psums (per head):
    #   av[0:65] = [V|1]^T probs  -> rows 0:64 numerator, row 64 rowsum
    # The finisher normalizes lane-aligned at partitions 0:64 and lane-shifts
    # the odd head's a^T to partitions 64:128 with a SBUF->SBUF DMA.
    DELAY = 4  # software-pipeline distance between scores/exp and AV use

    def att_qp(Q, p):
        qlo = Q * 512
        Jmax = 4 * Q + 3
        nJ = 4 * Q + 4
        ava = ps_av.tile([128, 512], f32, tag="ava", name="ava")
        avb = ps_av.tile([128, 512], f32, tag="avb", name="avb")
        pts = []
        for J in range(nJ + DELAY):
            if J < nJ:
                jlo = J * 128
                istart = max(jlo, qlo)
                w = qlo + 512 - istart
                pss = ps_s.tile([128, 1024], f32, tag="s", name="pss")
                for hh in range(2):
                    nc.tensor.matmul(
                        pss[:, hh * 512: hh * 512 + w],
                        lhsT=kt_sb[hh * 64:(hh + 1) * 64, p * 2048 + jlo: p * 2048 + jlo + 128],
                        rhs=qt_sb[hh * 64:(hh + 1) * 64, p * 2048 + istart: p * 2048 + istart + w],
                        start=True, stop=True)
                pt = ppt.tile([128, 1024], bf16, tag="pt", name="pt")
                nc.scalar.activation(
                    pt.rearrange("x (h c) -> x h c", c=512)[:, :, 0:w],
                    pss.rearrange("x (h c) -> x h c", c=512)[:, :, 0:w],
                    EXP, scale=SCALE)
                if jlo >= qlo:
                    # diagonal j-tile: zero the j > i triangle
                    nc.gpsimd.tensor_mul(pt[:, 0:128], pt[:, 0:128], tri_sb[:])
                    nc.gpsimd.tensor_mul(pt[:, 512:640], pt[:, 512:640], tri_sb[:])
                pts.append((pt, istart - qlo, w))
            if J == 0 and pend:
                pend.pop(0)()
            # (3,1): hold pops for the drain steps, where exp pacing bites
            if J % 2 == 0 and (J >= 6 or (Q, p) != (3, 1)):
                pop_filler(2 * Q + p)
            Ja = J - DELAY
            if Ja < 0:
                continue
            pt, co, w = pts[Ja]
            base = Ja * VBLK + p * 130
            kw = dict(start=(Ja == 0), stop=(Ja == Jmax), skip_group_check=True)
            nc.tensor.matmul(ava[0:65, co:512], lhsT=v_sb[:, base: base + 65],
                             rhs=pt[:, 0:w], **kw)
            nc.tensor.matmul(avb[0:65, co:512], lhsT=v_sb[:, base + 65: base + 130],
                             rhs=pt[:, 512:512 + w], **kw)

        def finish():
            ones_row = onesr[64:65, :]
            rec = prec.tile([128, 1024], f32r, tag="rec", name="rec")
            with nc.allow_low_precision(reason="softmax denominators in f32r"):
                nc.vector.reciprocal(rec[64:65, 0:512], ava[64:65, :])
                nc.vector.reciprocal(rec[64:65, 512:1024], avb[64:65, :])
            # broadcast each head's reciprocal row over 64 partitions; DVE may
            # read only one PSUM operand, so stage the broadcast in SBUF.
            bc_sb = prec.tile([128, 1024], f32r, tag="bcs", name="bc_sb")
            for half in range(2):
                bc = ps_mm.tile([128, 512], f32, tag="acc", name="bc")
                nc.tensor.matmul(bc[0:64, :], lhsT=ones_row,
                                 rhs=rec[64:65, half * 512:(half + 1) * 512],
                                 start=True, stop=True)
                if Q < 2:  # ACT has slack in the early quarters
                    nc.scalar.activation(
                        bc_sb[0:64, half * 512:(half + 1) * 512], bc[0:64, :],
                        CPY)
                else:
                    nc.vector.tensor_copy(
                        bc_sb[0:64, half * 512:(half + 1) * 512], bc[0:64, :])
            nc.vector.tensor_mul(
                at_sb[0:64, p * 2048 + qlo: p * 2048 + qlo + 512],
                ava[0:64, :], bc_sb[0:64, 0:512])
            at_tmp = prec.tile([128, 512], f32r, tag="att", name="at_tmp")
            nc.vector.tensor_mul(at_tmp[0:64, :], avb[0:64, :],
                                 bc_sb[0:64, 512:1024])
            if Q == 3 and p == 1:
                # the tail reads this tile directly (split contraction), so
                # the lane-shift DMA is off the closing critical path
                last_at.append(at_tmp)
            else:
                nc.sync.dma_start(
                    at_sb[64:128, p * 2048 + qlo: p * 2048 + qlo + 512],
                    at_tmp[0:64, :])
            if p == 1 and Q < 3:
                # Q's proj stiles: spread over the later, ACT-bound stretches
                keys = {0: (2, 2, 2, 2), 1: (4, 4, 5, 5), 2: (7, 7, 7, 7)}[Q]
                filler.extend(
                    (kk, proj_stile(st))
                    for kk, st in zip(keys, range(4 * Q, 4 * Q + 4)))
        pend.append(finish)

    for Q in range(4):
        att_qp(Q, 0)
        att_qp(Q, 1)
    while filler:
        filler.pop(0)[1]()
    # tail: overlap st12's pair-0 accumulation (PSUM held open in the freed
    # score slots) with the last finisher's normalize chain, then finish the
    # remaining stiles with ACT copies while DVE/DMA drain.
    split = []
    for ec in range(2):
        ps = ps_s.tile([128, 512], f32, tag="s", name="ps_f")
        nc.tensor.matmul(ps[:], lhsT=at_sb[:, 12 * 128: 13 * 128],
                         rhs=wp_sb[:, ec * 512:(ec + 1) * 512],
                         start=True, stop=False)
        split.append(ps)
    while pend:
        pend.pop(0)()
    att3 = last_at[0]

    def tail_k2(ps, st, ec):
        # pair-1 contraction split: even head from at_sb[0:64], odd head
        # straight from the finisher's temp tile (no lane-shift DMA)
        nc.tensor.matmul(
            ps[:], lhsT=at_sb[0:64, 2048 + st * 128: 2048 + (st + 1) * 128],
            rhs=wp_sb[0:64, 1024 + ec * 512: 1024 + (ec + 1) * 512],
            start=False, stop=False)
        nc.tensor.matmul(
            ps[:], lhsT=att3[0:64, (st - 12) * 128: (st - 11) * 128],
            rhs=wp2_sb[0:64, ec * 512:(ec + 1) * 512],
            start=False, stop=True)

    stage12 = pstage.tile([128, 1024], bf16, tag="stage", name="stage12")
    for ec in range(2):
        tail_k2(split[ec], 12, ec)
        nc.scalar.activation(stage12[:, ec * 512:(ec + 1) * 512],
                             split[ec][:], CPY)
    nc.sync.dma_start(out[12 * 128:13 * 128, :], stage12[:])
    for st in (13, 14, 15):
        stage = pstage.tile([128, 1024], bf16, tag="stage", name="stageT")
        for ec in range(2):
            ps = ps_mm.tile([128, 512], f32, tag="acc", name="ps_t")
            nc.tensor.matmul(ps[:], lhsT=at_sb[:, st * 128:(st + 1) * 128],
                             rhs=wp_sb[:, ec * 512:(ec + 1) * 512],
                             start=True, stop=False)
            tail_k2(ps, st, ec)
            nc.scalar.activation(stage[:, ec * 512:(ec + 1) * 512], ps[:], CPY)
            if st >= 14:  # per-half DMAs shorten the closing drain
                nc.sync.dma_start(
                    out[st * 128:(st + 1) * 128, ec * 512:(ec + 1) * 512],
                    stage[:, ec * 512:(ec + 1) * 512])
        if st < 14:
            nc.sync.dma_start(out[st * 128:(st + 1) * 128, :], stage[:])


def _build_nc(repeat=1):
    key = ("nc", repeat)
    if key in _CACHE:
        return _CACHE[key]
    import concourse.bacc as bacc
    import concourse.mybir as mybir
    import concourse.tile as tile

    f32 = mybir.dt.float32
    f32r = mybir.dt.float32r
    bf16d = mybir.dt.bfloat16
    nc = bacc.Bacc("TRN2", target_bir_lowering=False, debug=False)
    xt = nc.dram_tensor("xt", [D, S], bf16d, kind="ExternalInput").ap()
    wqk = nc.dram_tensor("wqk", [128, 4096], bf16d, kind="ExternalInput").ap()
    wv = nc.dram_tensor("wv", [128, 2048], bf16d, kind="ExternalInput").ap()
    wp = nc.dram_tensor("wp", [128, 2048], f32r, kind="ExternalInput").ap()
    wp2 = nc.dram_tensor("wp2", [128, 1024], f32r, kind="ExternalInput").ap()
    qkb = nc.dram_tensor("qkb", [128, 4], f32, kind="ExternalInput").ap()
    vb = nc.dram_tensor("vb", [128, 256], f32, kind="ExternalInput").ap()
    tri = nc.dram_tensor("tri", [128, 128], bf16d, kind="ExternalInput").ap()
    onesd = nc.dram_tensor("onesd", [128, 64], f32r, kind="ExternalInput").ap()
    onesh = nc.dram_tensor("onesh", [128, 64], bf16d, kind="ExternalInput").ap()
    out = nc.dram_tensor("out", [S, D], bf16d, kind="ExternalOutput").ap()

    with tile.TileContext(nc) as tc:
        for _ in range(repeat):
            with ExitStack() as ctx:
                _body(ctx, tc, mybir, xt, wqk, wv, wp, wp2, qkb, vb, tri, onesd, onesh, out)
    nc.compile()
    _CACHE[key] = nc
    return nc


def _make_in_maps(hidden_states, c_attn_w, c_attn_b, c_proj_w):
    hs = np.asarray(hidden_states, dtype=np.float32)
    waw = np.asarray(c_attn_w, dtype=np.float32)
    wab = np.asarray(c_attn_b, dtype=np.float32)
    wpw = np.asarray(c_proj_w, dtype=np.float32)

    tri = np.triu(np.ones((128, 128), dtype=ml_dtypes.bfloat16))
    xts = [np.ascontiguousarray(hs[b].T).astype(ml_dtypes.bfloat16) for b in range(B)]
    in_maps = []
    for c in range(NCORES):
        b, g = divmod(c, GROUPS)
        cols = np.arange(g * HPC * HD, (g + 1) * HPC * HD)
        wqk_host = np.concatenate([waw[:, cols], waw[:, D + cols]], axis=1)
        in_maps.append({
            "xt": xts[b],
            "wqk": np.ascontiguousarray(
                wqk_host.reshape(8, 128, 512).transpose(1, 0, 2).reshape(128, 4096)).astype(ml_dtypes.bfloat16),
            "wv": np.ascontiguousarray(
                waw[:, 2 * D + cols].reshape(8, 128, 256).transpose(1, 0, 2).reshape(128, 2048)).astype(ml_dtypes.bfloat16),
            "wp": np.ascontiguousarray(
                wpw[cols, :].reshape(2, 128, 1024).transpose(1, 0, 2).reshape(128, 2048)),
            "wp2": np.concatenate(
                [wpw[cols[192:256], :], np.zeros((64, D), np.float32)], axis=0),
            "qkb": np.ascontiguousarray(
                np.concatenate([wab[cols], wab[D + cols]]).reshape(4, 128).T),
            "vb": np.ascontiguousarray(
                np.broadcast_to(wab[2 * D + cols], (128, 256))),
            "tri": tri,
            "onesd": np.ones((128, 64), np.float32),
            "onesh": np.ones((128, 64), ml_dtypes.bfloat16),
        })
    return in_maps


def kernel(hidden_states, c_attn_w, c_attn_b, c_proj_w, c_proj_b):
    from concourse import bass_utils

    nc = _build_nc()
    in_maps = _make_in_maps(hidden_states, c_attn_w, c_attn_b, c_proj_w)
    res = bass_utils.run_bass_kernel_spmd(nc, in_maps, core_ids=list(range(NCORES)))
    outs = [np.asarray(r["out"], dtype=np.float32) for r in res.results]
    wpb = np.asarray(c_proj_b, dtype=np.float32)
    full = np.stack(
        [sum(outs[b * GROUPS:(b + 1) * GROUPS]) + wpb for b in range(B)], axis=0)
    return full.astype(np.float32)


# revision 62
# speedup vs baseline: 1.4879x; 1.0035x over previous
"""GPT-2-style causal attention block on 8 TRN2 NeuronCores (Bass/Tile).

Sharding (Megatron-style, per the hint): core c handles batch b = c // 4 and
head-group g = c % 4 (4 of the 16 heads).  Each core computes, fully locally:
  QKV projection (its 4 heads' columns), causal softmax attention for its
  4 heads, and the row-sharded output projection partial [S, D].
The host gathers by summing the 4 partials per batch and adding c_proj_b.

Per-core kernel layout choices:
  - x^T [D, S] is staged on host so Q^T/K^T come out of matmuls directly with
    head_dim on partitions (what the scores matmul wants) and V comes out in
    [seq, head_dim] (what the AV matmul wants).
  - scores are computed transposed, sT[j, i] (j = key index on partitions), so
    the exp'd tile is directly usable as the AV matmul's moving operand.
  - softmax denominator comes from the SAME matmul as AV: each head's V block
    carries a ones column ([V|1]), so psum row 64 is the rowsum and rows 0:64
    the numerator.  Normalization: DVE reciprocal of row 64, a 1-contraction
    matmul against a ones row (reused from tri) broadcasts it over 64
    partitions, then a DVE mul writes a^T.  The odd head's product lands in a
    temp tile at partitions 0:64 and is lane-shifted to at[64:128] by a tiny
    SBUF->SBUF DMA.  This halves attention PE work vs a separate ones-matmul.
  - QKV phase A runs k-outer while x^T/W stream from HBM: Q/K (i-halves
    sc0, sc1) and V (j0-3) accumulate in 8 live PSUM banks, so PE tracks the
    input DMA instead of idling.  x^T lands in 512-col quarters for finer
    pipelining.  The rest of QKV + the output projection are emitted as
    "filler" work units between attention J-steps, keeping PE busy while the
    exp stream paces softmax.
  - x/Wqkv/scores/probs/V run in bf16; the output projection in float32r.
"""

from contextlib import ExitStack

import ml_dtypes
import numpy as np

B, S, D = 2, 2048, 1024
NH, HD = 16, 64
NCORES = 8
GROUPS = 4           # tensor-parallel head groups per batch
HPC = NH // GROUPS   # heads per core
SCALE = 1.0 / 8.0    # 1/sqrt(HD)
VBLK = 260           # per-j-tile V block: [V0|1|V1|1|V2|1|V3|1]

_CACHE = {}


def _body(ctx, tc, mybir, xt, wqk, wv, wp, wp2, qkb, vb, tri, onesd, onesh, out):
    nc = tc.nc
    f32 = mybir.dt.float32
    f32r = mybir.dt.float32r
    bf16 = mybir.dt.bfloat16
    EXP = mybir.ActivationFunctionType.Exp

    pin = ctx.enter_context(tc.tile_pool(name="pin", bufs=1))
    pwork = ctx.enter_context(tc.tile_pool(name="pwork", bufs=1))
    ppt = ctx.enter_context(tc.tile_pool(name="ppt", bufs=12))
    prec = ctx.enter_context(tc.tile_pool(name="prec", bufs=6))
    pstage = ctx.enter_context(tc.tile_pool(name="pstage", bufs=6))
    ps_mm = ctx.enter_context(tc.tile_pool(name="ps_mm", bufs=2, space="PSUM"))
    ps_s = ctx.enter_context(tc.tile_pool(name="ps_s", bufs=2, space="PSUM"))
    ps_av = ctx.enter_context(tc.tile_pool(name="ps_av", bufs=1, space="PSUM"))

    # ---------------- input staging ----------------
    # Each DMA costs ~625ns of serial HWDGE time on top of its transfer, so
    # batch inputs into few, large DMAs ordered by first use: the first wqk
    # block + x^T chunk 0 unblock the first phase-A matmul ASAP, small
    # constants slot into the stream just before their first consumer.
    xt_sb = pin.tile([128, 8 * 2048], bf16, name="xt_sb")
    wqk_sb = pin.tile([128, 4096], bf16, name="wqk_sb")
    wv_sb = pin.tile([128, 2048], bf16, name="wv_sb")
    nc.sync.dma_start(wqk_sb[:, 0:512], wqk[:, 0:512])
    nc.sync.dma_start(xt_sb[:, 0:512], xt[0:128, 0:512])
    nc.sync.dma_start(wv_sb[:], wv[:])
    nc.sync.dma_start(xt_sb[:, 512:2048], xt[0:128, 512:2048])
    for k in range(1, 8):
        nc.sync.dma_start(wqk_sb[:, k * 512:(k + 1) * 512],
                          wqk[:, k * 512:(k + 1) * 512])
        nc.sync.dma_start(xt_sb[:, k * 2048:(k + 1) * 2048],
                          xt[k * 128:(k + 1) * 128, :])
    qkb_sb = pin.tile([128, 4], f32, name="qkb_sb")
    nc.sync.dma_start(qkb_sb[:], qkb[:])
    tri_sb = pin.tile([128, 128], bf16, name="tri_sb")
    nc.sync.dma_start(tri_sb[:], tri[:])
    vb_sb = pin.tile([128, 256], f32, name="vb_sb")
    nc.sync.dma_start(vb_sb[:], vb[:])
    onesr = pin.tile([128, 64], f32r, name="onesr")
    nc.sync.dma_start(onesr[:], onesd[:])
    ones64 = pin.tile([128, 64], bf16, name="ones64")
    nc.sync.dma_start(ones64[:], onesh[:])
    wp_sb = pin.tile([128, 2048], f32r, name="wp_sb")
    nc.sync.dma_start(wp_sb[:], wp[:])
    wp2_sb = pin.tile([128, 1024], f32r, name="wp2_sb")
    nc.sync.dma_start(wp2_sb[:], wp2[:])

    # Q^T / K^T: head-pair p at cols [p*2048, (p+1)*2048); head hh of the pair
    # on partitions [hh*64, hh*64+64).
    qt_sb = pwork.tile([128, 2 * 2048], bf16, name="qt_sb")
    kt_sb = pwork.tile([128, 2 * 2048], bf16, name="kt_sb")
    # V blocks of VBLK cols per j-tile: head h's [V_h|1] at cols h*65
    v_sb = pwork.tile([128, 16 * VBLK], bf16, name="v_sb")
    # a^T: k2 (head pair) at cols [k2*2048, ...), head hh on partitions hh*64..
    at_sb = pwork.tile([128, 2 * 2048], f32r, name="at_sb")

    # Dummy exp so the ACT table set loads during the input-DMA window instead
    # of delaying the first real softmax exp (wv lands within ~5us).
    warm = pin.tile([128, 4], f32, name="warm")
    nc.scalar.activation(warm[:], wv_sb[:, 0:4], EXP, scale=0.0)

    # ---------------- QKV helpers ----------------
    # col-tiles: C=0 -> Q pair0, C=1 -> Q pair1, C=2 -> K pair0, C=3 -> K pair1
    # psum->SBUF bias-add moves run on ACT (Copy is in the exp table set, and
    # ACT is idle during the QKV phases) to keep DVE free for softmax work.
    CPY = mybir.ActivationFunctionType.Copy
    IDN = mybir.ActivationFunctionType.Identity

    def qk_add(ps_ap, C, sc, on_act=True):
        dest = qt_sb if C < 2 else kt_sb
        p = C % 2
        dst = dest[:, p * 2048 + sc * 512: p * 2048 + (sc + 1) * 512]
        if on_act:
            nc.scalar.activation(dst, ps_ap, IDN, bias=qkb_sb[:, C:C + 1])
        else:  # late blocks run inside ACT-paced attention windows
            nc.vector.tensor_scalar_add(dst, ps_ap, qkb_sb[:, C:C + 1])

    def v_add(ps_ap256, j):
        dst = v_sb[:, j * VBLK:(j + 1) * VBLK].rearrange(
            "p (g c) -> p g c", c=65)[:, :, 0:64]
        nc.vector.tensor_add(
            dst,
            ps_ap256.rearrange("p (g c) -> p g c", c=64),
            vb_sb.rearrange("p (g c) -> p g c", c=64))

    # ---------------- phase A: k-outer QKV subset ----------------
    # All 8 PSUM banks track the input stream: Q/K for sc0 (acc0/acc1),
    # sc1 (ava/avb), sc2 (packed 2-bank s tile), V j0-3 (packed s tile).
    a_q0 = ps_mm.tile([128, 512], f32, tag="acc", name="a_q0")
    a_k0 = ps_mm.tile([128, 512], f32, tag="acc", name="a_k0")
    a_q1 = ps_av.tile([128, 512], f32, tag="ava", name="a_q1")
    a_k1 = ps_av.tile([128, 512], f32, tag="avb", name="a_k1")
    a_v = ps_s.tile([128, 1024], f32, tag="s", name="a_v")
    a_qk2 = ps_s.tile([128, 1024], f32, tag="s", name="a_qk2")
    for k in range(8):
        for (ps_t, C, sc) in ((a_q0, 0, 0), (a_k0, 2, 0)):
            nc.tensor.matmul(
                ps_t[:],
                lhsT=wqk_sb[:, k * 512 + C * 128: k * 512 + (C + 1) * 128],
                rhs=xt_sb[:, k * 2048 + sc * 512: k * 2048 + (sc + 1) * 512],
                start=(k == 0), stop=(k == 7))
        for j in range(4):
            # start=True zeroes the whole 2KB bank region, so only the first
            # group per bank (j=0 for cols 0:512, j=2 for 512:1024) may start.
            nc.tensor.matmul(
                a_v[:, j * 256:(j + 1) * 256],
                lhsT=xt_sb[:, k * 2048 + j * 128: k * 2048 + (j + 1) * 128],
                rhs=wv_sb[:, k * 256:(k + 1) * 256],
                start=(k == 0 and j % 2 == 0), stop=(k == 7),
                skip_group_check=True)
        for (ps_t, C, sc) in ((a_q1, 0, 1), (a_k1, 2, 1)):
            nc.tensor.matmul(
                ps_t[:],
                lhsT=wqk_sb[:, k * 512 + C * 128: k * 512 + (C + 1) * 128],
                rhs=xt_sb[:, k * 2048 + sc * 512: k * 2048 + (sc + 1) * 512],
                start=(k == 0), stop=(k == 7))
        for (co, C) in ((0, 0), (512, 2)):  # sc2, one group per bank
            nc.tensor.matmul(
                a_qk2[:, co:co + 512],
                lhsT=wqk_sb[:, k * 512 + C * 128: k * 512 + (C + 1) * 128],
                rhs=xt_sb[:, k * 2048 + 1024: k * 2048 + 1536],
                start=(k == 0), stop=(k == 7))
    qk_add(a_k0[:], 2, 0)
    qk_add(a_q0[:], 0, 0)
    for j in range(4):
        v_add(a_v[:, j * 256:(j + 1) * 256], j)
    qk_add(a_q1[:], 0, 1)
    qk_add(a_k1[:], 2, 1)
    qk_add(a_qk2[:, 0:512], 0, 2)
    qk_add(a_qk2[:, 512:1024], 2, 2)
    # ones columns of the V blocks (emitted after the v_adds so the DVE queue
    # is not head-blocked waiting for the onesh DMA; disjoint columns)
    nc.vector.tensor_copy(
        v_sb.rearrange("p (g c) -> p g c", c=65)[:, :, 64:65],
        ones64.rearrange("p (g c) -> p g c", c=1))

    # ---------------- filler work units ----------------
    def qk_block(sc, C, on_act=True):
        def go():
            ps = ps_mm.tile([128, 512], f32, tag="acc", name="qkB")
            for k in range(8):
                nc.tensor.matmul(
                    ps[:],
                    lhsT=wqk_sb[:, k * 512 + C * 128: k * 512 + (C + 1) * 128],
                    rhs=xt_sb[:, k * 2048 + sc * 512: k * 2048 + (sc + 1) * 512],
                    start=(k == 0), stop=(k == 7))
            qk_add(ps[:], C, sc, on_act=on_act)
        return go

    def v_block(j):
        def go():
            ps = ps_mm.tile([128, 256], f32, tag="acc", name="vB")
            for k in range(8):
                nc.tensor.matmul(
                    ps[:],
                    lhsT=xt_sb[:, k * 2048 + j * 128: k * 2048 + (j + 1) * 128],
                    rhs=wv_sb[:, k * 256:(k + 1) * 256],
                    start=(k == 0), stop=(k == 7))
            v_add(ps[:], j)
        return go

    def proj_stile(st, on_act=False):
        def go():
            stage = pstage.tile([128, 1024], bf16, tag="stage", name="stage")
            for ec in range(2):
                ps = ps_mm.tile([128, 512], f32, tag="acc", name="ps_o")
                for k2 in range(2):
                    nc.tensor.matmul(
                        ps[:],
                        lhsT=at_sb[:, k2 * 2048 + st * 128: k2 * 2048 + (st + 1) * 128],
                        rhs=wp_sb[:, k2 * 1024 + ec * 512: k2 * 1024 + (ec + 1) * 512],
                        start=(k2 == 0), stop=(k2 == 1))
                if on_act:  # tail stiles: ACT is idle once the exps are done
                    nc.scalar.activation(stage[:, ec * 512:(ec + 1) * 512],
                                         ps[:], CPY)
                else:
                    nc.vector.tensor_copy(stage[:, ec * 512:(ec + 1) * 512],
                                          ps[:])
            nc.sync.dma_start(out[st * 128:(st + 1) * 128, :], stage[:])
        return go

    # Filler queue: (key, closure) where key = 2*Q + p of the earliest
    # attention instance allowed to pop it.  Reserving late work for the
    # ACT-saturated Q2/Q3 stretches keeps PE fed end-to-end.
    filler = [(0, qk_block(0, 1)), (0, qk_block(0, 3)),
              (0, qk_block(1, 1)), (0, qk_block(1, 3)),
              (1, v_block(4)), (1, v_block(5)),
              (1, v_block(6)), (1, v_block(7)),
              (2, qk_block(3, 0, False)), (2, qk_block(3, 2, False)),
              (2, v_block(8)), (2, v_block(9)),
              (3, v_block(10)), (3, v_block(11)),
              (4, qk_block(2, 1, False)), (4, qk_block(2, 3, False)),
              (6, qk_block(3, 1, False)), (6, qk_block(3, 3, False)),
              (6, v_block(12)), (6, v_block(13)),
              (6, v_block(14)), (6, v_block(15))]

    def pop_filler(key):
        for i, (mk, go) in enumerate(filler):
            if mk <= key:
                filler.pop(i)
                go()
                return

    pend = []  # deferred attention finishers (normalize + a^T write)
    last_at = []  # (3,1)'s odd-head a^T temp, consumed by the tail

    # ---------------- attention ----------------
    # Per (i-quarter Q of 512, head-pair p).  Scores for both heads of the
    # pair share one [128, 1024] PSUM tile (head hh at cols hh*512) so one
    # strided exp covers both.  AV psums (per head):
    #   av[0:65] = [V|1]^T probs  -> rows 0:64 numerator, row 64 rowsum
    # The finisher normalizes lane-aligned at partitions 0:64 and lane-shifts
    # the odd head's a^T to partitions 64:128 with a SBUF->SBUF DMA.
    DELAY = 4  # software-pipeline distance between scores/exp and AV use

    def att_qp(Q, p):
        qlo = Q * 512
        Jmax = 4 * Q + 3
        nJ = 4 * Q + 4
        ava = ps_av.tile([128, 512], f32, tag="ava", name="ava")
        avb = ps_av.tile([128, 512], f32, tag="avb", name="avb")
        pts = []
        # Diagonal j-tiles first: PSUM accumulation is commutative, and this
        # moves the Pool tri-masks off the drain-phase critical path (the
        # closing AVs then consume mask-free full tiles).
        jorder = list(range(4 * Q, nJ)) + list(range(0, 4 * Q))
        for idx in range(nJ + DELAY):
            if idx < nJ:
                jlo = jorder[idx] * 128
                istart = max(jlo, qlo)
                w = qlo + 512 - istart
                pss = ps_s.tile([128, 1024], f32, tag="s", name="pss")
                for hh in range(2):
                    nc.tensor.matmul(
                        pss[:, hh * 512: hh * 512 + w],
                        lhsT=kt_sb[hh * 64:(hh + 1) * 64, p * 2048 + jlo: p * 2048 + jlo + 128],
                        rhs=qt_sb[hh * 64:(hh + 1) * 64, p * 2048 + istart: p * 2048 + istart + w],
                        start=True, stop=True)
                pt = ppt.tile([128, 1024], bf16, tag="pt", name="pt")
                nc.scalar.activation(
                    pt.rearrange("x (h c) -> x h c", c=512)[:, :, 0:w],
                    pss.rearrange("x (h c) -> x h c", c=512)[:, :, 0:w],
                    EXP, scale=SCALE)
                if jlo >= qlo:
                    # diagonal j-tile: zero the j > i triangle
                    nc.gpsimd.tensor_mul(pt[:, 0:128], pt[:, 0:128], tri_sb[:])
                    nc.gpsimd.tensor_mul(pt[:, 512:640], pt[:, 512:640], tri_sb[:])
                pts.append((pt, istart - qlo, w))
            if idx == 0 and pend:
                pend.pop(0)()
            # Q3: force the urgent pops early (v12-15 must beat the first
            # AVs), then hold the rest for the ACT-bound drain steps.
            if (Q, p) == (3, 0):
                if idx < 6 or idx >= 16:
                    pop_filler(6)
            elif (Q, p) == (3, 1):
                if idx >= 13:
                    pop_filler(7)
            elif idx % 2 == 0:
                pop_filler(2 * Q + p)
            ia = idx - DELAY
            if ia < 0:
                continue
            pt, co, w = pts[ia]
            base = jorder[ia] * VBLK + p * 130
            kw = dict(start=(ia == 0), stop=(ia == Jmax), skip_group_check=True)
            nc.tensor.matmul(ava[0:65, co:512], lhsT=v_sb[:, base: base + 65],
                             rhs=pt[:, 0:w], **kw)
            nc.tensor.matmul(avb[0:65, co:512], lhsT=v_sb[:, base + 65: base + 130],
                             rhs=pt[:, 512:512 + w], **kw)

        def finish():
            ones_row = onesr[64:65, :]
            rec = prec.tile([128, 1024], f32r, tag="rec", name="rec")
            with nc.allow_low_precision(reason="softmax denominators in f32r"):
                nc.vector.reciprocal(rec[64:65, 0:512], ava[64:65, :])
                nc.vector.reciprocal(rec[64:65, 512:1024], avb[64:65, :])
            # broadcast each head's reciprocal row over 64 partitions; DVE may
            # read only one PSUM operand, so stage the broadcast in SBUF.
            bc_sb = prec.tile([128, 1024], f32r, tag="bcs", name="bc_sb")
            for half in range(2):
                bc = ps_mm.tile([128, 512], f32, tag="acc", name="bc")
                nc.tensor.matmul(bc[0:64, :], lhsT=ones_row,
                                 rhs=rec[64:65, half * 512:(half + 1) * 512],
                                 start=True, stop=True)
                if Q < 2:  # ACT has slack in the early quarters
                    nc.scalar.activation(
                        bc_sb[0:64, half * 512:(half + 1) * 512], bc[0:64, :],
                        CPY)
                else:
                    nc.vector.tensor_copy(
                        bc_sb[0:64, half * 512:(half + 1) * 512], bc[0:64, :])
            nc.vector.tensor_mul(
                at_sb[0:64, p * 2048 + qlo: p * 2048 + qlo + 512],
                ava[0:64, :], bc_sb[0:64, 0:512])
            at_tmp = prec.tile([128, 512], f32r, tag="att", name="at_tmp")
            nc.vector.tensor_mul(at_tmp[0:64, :], avb[0:64, :],
                                 bc_sb[0:64, 512:1024])
            if Q == 3 and p == 1:
                # the tail reads this tile directly (split contraction), so
                # the lane-shift DMA is off the closing critical path
                last_at.append(at_tmp)
            else:
                nc.sync.dma_start(
                    at_sb[64:128, p * 2048 + qlo: p * 2048 + qlo + 512],
                    at_tmp[0:64, :])
            if p == 1 and Q < 3:
                # Q's proj stiles: spread over the later, ACT-bound stretches
                keys = {0: (2, 2, 2, 2), 1: (4, 4, 5, 5), 2: (6, 6, 7, 7)}[Q]
                filler.extend(
                    (kk, proj_stile(st))
                    for kk, st in zip(keys, range(4 * Q, 4 * Q + 4)))
        pend.append(finish)

    for Q in range(4):
        att_qp(Q, 0)
        att_qp(Q, 1)
    while filler:
        filler.pop(0)[1]()
    # tail: overlap st12's pair-0 accumulation (PSUM held open in the freed
    # score slots) with the last finisher's normalize chain, then finish the
    # remaining stiles with ACT copies while DVE/DMA drain.
    split = []
    for ec in range(2):
        ps = ps_s.tile([128, 512], f32, tag="s", name="ps_f")
        nc.tensor.matmul(ps[:], lhsT=at_sb[:, 12 * 128: 13 * 128],
                         rhs=wp_sb[:, ec * 512:(ec + 1) * 512],
                         start=True, stop=False)
        split.append(ps)
    while pend:
        pend.pop(0)()
    att3 = last_at[0]

    def tail_k2(ps, st, ec):
        # pair-1 contraction split: even head from at_sb[0:64], odd head
        # straight from the finisher's temp tile (no lane-shift DMA)
        nc.tensor.matmul(
            ps[:], lhsT=at_sb[0:64, 2048 + st * 128: 2048 + (st + 1) * 128],
            rhs=wp_sb[0:64, 1024 + ec * 512: 1024 + (ec + 1) * 512],
            start=False, stop=False)
        nc.tensor.matmul(
            ps[:], lhsT=att3[0:64, (st - 12) * 128: (st - 11) * 128],
            rhs=wp2_sb[0:64, ec * 512:(ec + 1) * 512],
            start=False, stop=True)

    stage12 = pstage.tile([128, 1024], bf16, tag="stage", name="stage12")
    for ec in range(2):
        tail_k2(split[ec], 12, ec)
        nc.scalar.activation(stage12[:, ec * 512:(ec + 1) * 512],
                             split[ec][:], CPY)
    nc.sync.dma_start(out[12 * 128:13 * 128, :], stage12[:])
    for st in (13, 14, 15):
        stage = pstage.tile([128, 1024], bf16, tag="stage", name="stageT")
        for ec in range(2):
            ps = ps_mm.tile([128, 512], f32, tag="acc", name="ps_t")
            nc.tensor.matmul(ps[:], lhsT=at_sb[:, st * 128:(st + 1) * 128],
                             rhs=wp_sb[:, ec * 512:(ec + 1) * 512],
                             start=True, stop=False)
            tail_k2(ps, st, ec)
            nc.scalar.activation(stage[:, ec * 512:(ec + 1) * 512], ps[:], CPY)
            if st >= 14:  # per-half DMAs shorten the closing drain
                nc.sync.dma_start(
                    out[st * 128:(st + 1) * 128, ec * 512:(ec + 1) * 512],
                    stage[:, ec * 512:(ec + 1) * 512])
        if st < 14:
            nc.sync.dma_start(out[st * 128:(st + 1) * 128, :], stage[:])


def _build_nc(repeat=1):
    key = ("nc", repeat)
    if key in _CACHE:
        return _CACHE[key]
    import concourse.bacc as bacc
    import concourse.mybir as mybir
    import concourse.tile as tile

    f32 = mybir.dt.float32
    f32r = mybir.dt.float32r
    bf16d = mybir.dt.bfloat16
    nc = bacc.Bacc("TRN2", target_bir_lowering=False, debug=False)
    xt = nc.dram_tensor("xt", [D, S], bf16d, kind="ExternalInput").ap()
    wqk = nc.dram_tensor("wqk", [128, 4096], bf16d, kind="ExternalInput").ap()
    wv = nc.dram_tensor("wv", [128, 2048], bf16d, kind="ExternalInput").ap()
    wp = nc.dram_tensor("wp", [128, 2048], f32r, kind="ExternalInput").ap()
    wp2 = nc.dram_tensor("wp2", [128, 1024], f32r, kind="ExternalInput").ap()
    qkb = nc.dram_tensor("qkb", [128, 4], f32, kind="ExternalInput").ap()
    vb = nc.dram_tensor("vb", [128, 256], f32, kind="ExternalInput").ap()
    tri = nc.dram_tensor("tri", [128, 128], bf16d, kind="ExternalInput").ap()
    onesd = nc.dram_tensor("onesd", [128, 64], f32r, kind="ExternalInput").ap()
    onesh = nc.dram_tensor("onesh", [128, 64], bf16d, kind="ExternalInput").ap()
    out = nc.dram_tensor("out", [S, D], bf16d, kind="ExternalOutput").ap()

    with tile.TileContext(nc) as tc:
        for _ in range(repeat):
            with ExitStack() as ctx:
                _body(ctx, tc, mybir, xt, wqk, wv, wp, wp2, qkb, vb, tri, onesd, onesh, out)
    nc.compile()
    _CACHE[key] = nc
    return nc


def _make_in_maps(hidden_states, c_attn_w, c_attn_b, c_proj_w):
    hs = np.asarray(hidden_states, dtype=np.float32)
    waw = np.asarray(c_attn_w, dtype=np.float32)
    wab = np.asarray(c_attn_b, dtype=np.float32)
    wpw = np.asarray(c_proj_w, dtype=np.float32)

    tri = np.triu(np.ones((128, 128), dtype=ml_dtypes.bfloat16))
    xts = [np.ascontiguousarray(hs[b].T).astype(ml_dtypes.bfloat16) for b in range(B)]
    in_maps = []
    for c in range(NCORES):
        b, g = divmod(c, GROUPS)
        cols = np.arange(g * HPC * HD, (g + 1) * HPC * HD)
        wqk_host = np.concatenate([waw[:, cols], waw[:, D + cols]], axis=1)
        in_maps.append({
            "xt": xts[b],
            "wqk": np.ascontiguousarray(
                wqk_host.reshape(8, 128, 512).transpose(1, 0, 2).reshape(128, 4096)).astype(ml_dtypes.bfloat16),
            "wv": np.ascontiguousarray(
                waw[:, 2 * D + cols].reshape(8, 128, 256).transpose(1, 0, 2).reshape(128, 2048)).astype(ml_dtypes.bfloat16),
            "wp": np.ascontiguousarray(
                wpw[cols, :].reshape(2, 128, 1024).transpose(1, 0, 2).reshape(128, 2048)),
            "wp2": np.concatenate(
                [wpw[cols[192:256], :], np.zeros((64, D), np.float32)], axis=0),
            "qkb": np.ascontiguousarray(
                np.concatenate([wab[cols], wab[D + cols]]).reshape(4, 128).T),
            "vb": np.ascontiguousarray(
                np.broadcast_to(wab[2 * D + cols], (128, 256))),
            "tri": tri,
            "onesd": np.ones((128, 64), np.float32),
            "onesh": np.ones((128, 64), ml_dtypes.bfloat16),
        })
    return in_maps


def kernel(hidden_states, c_attn_w, c_attn_b, c_proj_w, c_proj_b):
    from concourse import bass_utils

    nc = _build_nc()
    in_maps = _make_in_maps(hidden_states, c_attn_w, c_attn_b, c_proj_w)
    res = bass_utils.run_bass_kernel_spmd(nc, in_maps, core_ids=list(range(NCORES)))
    outs = [np.asarray(r["out"], dtype=np.float32) for r in res.results]
    wpb = np.asarray(c_proj_b, dtype=np.float32)
    full = np.stack(
        [sum(outs[b * GROUPS:(b + 1) * GROUPS]) + wpb for b in range(B)], axis=0)
    return full.astype(np.float32)
